# revision 20
# baseline (speedup 1.0000x reference)
"""Trainium2 Bass kernel for nn_ArithmeticUserStateModel.

GRU-based user-state model: B=4096 users x T=256 sequential steps.
Pure data parallel across 8 NeuronCores (512 users per core, weights
replicated). All compute in a transposed layout: feature dims on SBUF
partitions, the 512 local users on the free dim.

Algebraic folding (host-side, exact):
  gates_x = (concat(pe, obs) @ proj_w + proj_b) @ gru_wih + gru_bih
          = peAug @ (proj_w @ gru_wih) + (proj_b @ gru_wih + gru_bih)
so the 34->64 projection matmul never runs on device.
"""

import sys

for _p in ("/opt/trn_rl_repo", "/opt/pypackages"):
    if _p not in sys.path:
        sys.path.insert(0, _p)

import numpy as np

import concourse.bacc as bacc
import concourse.tile as tile
from concourse import mybir
from concourse.bass_utils import run_bass_kernel_spmd

B, T = 4096, 256
NCORES = 8
BL = B // NCORES  # 512 users per core
PD, SD, NB = 32, 64, 41
TC = 8  # time chunk (steps per pipelined chunk)
F32 = mybir.dt.float32
AF = mybir.ActivationFunctionType
ALU = mybir.AluOpType

_CACHE = {}
TRACE_DIR = "/tmp/bass_trace"


def _build_nc():
    nc = bacc.Bacc(debug=False)

    probs = nc.declare_dram_parameter("probsT", [3, T, BL], F32, isOutput=False)
    obs = nc.declare_dram_parameter("obsT", [2, T, BL], F32, isOutput=False)

    wspec = {
        "w_e1": [3, 32], "b_e1": [32, 1],
        "w_e2": [32, 32], "b_e2": [32, 1],
        "w_xzr": [34, 128], "w_xn": [34, 64],
        "w_hzr": [64, 128], "w_hn": [64, 64],
        "b_zr": [128, 1], "b_hn_full": [128, 1], "b_n": [64, 1],
        "w_p1h": [64, 64], "w_p1pe": [32, 64], "b_p1": [64, 1],
        "w_p2": [64, 64], "b_p2": [64, 1],
        "w_po": [64, 42], "b_po": [42, 1],
    }
    wd = {k: nc.declare_dram_parameter(k, s, F32, isOutput=False)
          for k, s in wspec.items()}

    out_d = nc.declare_dram_parameter("out", [T, 42, BL], F32, isOutput=True)

    NCH = T // TC

    with tile.TileContext(nc) as tc:
        with (
            tc.tile_pool(name="const", bufs=1) as cpool,
            tc.tile_pool(name="probs", bufs=2) as prob_pool,
            tc.tile_pool(name="peaug", bufs=2) as pe_pool,
            tc.tile_pool(name="states", bufs=3) as st_pool,
            tc.tile_pool(name="work", bufs=2) as work,
            tc.tile_pool(name="outp", bufs=4) as opool,
            tc.tile_pool(name="ps_pe", bufs=2, space="PSUM") as ps_pe,
            tc.tile_pool(name="ps_rz", bufs=2, space="PSUM") as ps_rz,
            tc.tile_pool(name="ps_n", bufs=1, space="PSUM") as ps_n,
            tc.tile_pool(name="ps_pred", bufs=2, space="PSUM") as ps_pred,
        ):
            # ---- load weights/biases to SBUF once ----
            wt = {}
            for k, s in wspec.items():
                t_ = cpool.tile(s, F32, tag=k)
                nc.sync.dma_start(out=t_[:], in_=wd[k][:])
                wt[k] = t_

            st_cur = st_pool.tile([64, (TC + 1) * BL], F32, tag="states")
            nc.vector.memset(st_cur[:, 0:BL], 0.0)  # h_{-1} = 0

            for c in range(NCH):
                t0 = c * TC
                # ---------- Phase A: encoder + peAug chunk ----------
                probT = prob_pool.tile([3, TC * BL], F32, tag="probT")
                nc.sync.dma_start(
                    out=probT[:],
                    in_=probs[:, t0:t0 + TC, :],
                )
                peaug = pe_pool.tile([34, TC * BL], F32, tag="peaug")
                nc.sync.dma_start(
                    out=peaug[32:34, :],
                    in_=obs[:, t0:t0 + TC, :],
                )
                for tl in range(TC):
                    sl = slice(tl * BL, (tl + 1) * BL)
                    pe1p = ps_pe.tile([32, BL], F32, tag="psenc")
                    nc.tensor.matmul(pe1p[:], wt["w_e1"][:], probT[:, sl],
                                     start=True, stop=True)
                    pe1s = work.tile([32, BL], F32, tag="pe1s")
                    nc.scalar.activation(pe1s[:], pe1p[:], AF.Relu,
                                         bias=wt["b_e1"][:], scale=1.0)
                    pe2p = ps_pe.tile([32, BL], F32, tag="psenc")
                    nc.tensor.matmul(pe2p[:], wt["w_e2"][:], pe1s[:],
                                     start=True, stop=True)
                    nc.scalar.activation(peaug[0:32, sl], pe2p[:], AF.Relu,
                                         bias=wt["b_e2"][:], scale=1.0)

                st_next = st_pool.tile([64, (TC + 1) * BL], F32, tag="states")

                # ---------- Phase B+C: recurrence + prediction ----------
                for tl in range(TC):
                    t = t0 + tl
                    sl = slice(tl * BL, (tl + 1) * BL)
                    h_prev = st_cur[:, tl * BL:(tl + 1) * BL]

                    # gate pre-activations; column order is [z | r] so that
                    # z sits at base partition 0 (SBUF-SBUF ops need equal
                    # base partitions; r is only ever paired with PSUM).
                    prz = ps_rz.tile([128, BL], F32, tag="prz")
                    nc.tensor.matmul(prz[:], wt["w_xzr"][:], peaug[:, sl],
                                     start=True, stop=False)
                    nc.tensor.matmul(prz[:], wt["w_hzr"][:], h_prev,
                                     start=False, stop=True)
                    pxn = ps_n.tile([64, BL], F32, tag="pxn")
                    nc.tensor.matmul(pxn[:], wt["w_xn"][:], peaug[:, sl],
                                     start=True, stop=True)
                    # hn lands at psum base partition 64 so the stt below has
                    # base-aligned inputs (r lives at partitions 64:128).
                    phn = ps_n.tile([128, BL], F32, tag="phn")
                    nc.tensor.matmul(phn[64:128, :], wt["w_hn"][:], h_prev,
                                     start=True, stop=True)

                    zr = work.tile([128, BL], F32, tag="zr")
                    nc.scalar.activation(zr[:], prz[:], AF.Sigmoid,
                                         bias=wt["b_zr"][:], scale=1.0)
                    # rhn = (hn + b_hn) * r   (phn is PSUM, so the base-
                    # partition-64 SBUF operand r is allowed)
                    rhn = work.tile([64, BL], F32, tag="rhn")
                    nc.vector.scalar_tensor_tensor(
                        rhn[:], phn[64:128, :], wt["b_hn_full"][64:128, :],
                        zr[64:128, :],
                        op0=ALU.add, op1=ALU.mult)
                    sN = work.tile([64, BL], F32, tag="sN")
                    nc.vector.tensor_add(sN[:], rhn[:], pxn[:])
                    nT = work.tile([64, BL], F32, tag="nT")
                    nc.scalar.activation(nT[:], sN[:], AF.Tanh,
                                         bias=wt["b_n"][:], scale=1.0)
                    # h' = n + z*(h - n)
                    dT = work.tile([64, BL], F32, tag="dT")
                    nc.gpsimd.tensor_sub(dT[:], h_prev, nT[:])
                    eT = work.tile([64, BL], F32, tag="eT")
                    nc.gpsimd.tensor_mul(eT[:], zr[0:64, :], dT[:])
                    if tl < TC - 1:
                        h_dst = st_cur[:, (tl + 1) * BL:(tl + 2) * BL]
                    else:
                        h_dst = st_next[:, 0:BL]
                    nc.gpsimd.tensor_add(h_dst, nT[:], eT[:])

                    # prediction head (uses h_prev)
                    pf1 = ps_pred.tile([64, BL], F32, tag="pspred")
                    nc.tensor.matmul(pf1[:], wt["w_p1h"][:], h_prev,
                                     start=True, stop=False)
                    nc.tensor.matmul(pf1[:], wt["w_p1pe"][:], peaug[0:32, sl],
                                     start=False, stop=True)
                    f1 = work.tile([64, BL], F32, tag="f1")
                    nc.vector.tensor_scalar(f1[:], pf1[:], wt["b_p1"][:], 0.0,
                                            ALU.add, ALU.max)
                    pf2 = ps_pred.tile([64, BL], F32, tag="pspred")
                    nc.tensor.matmul(pf2[:], wt["w_p2"][:], f1[:],
                                     start=True, stop=True)
                    f2 = work.tile([64, BL], F32, tag="f2")
                    nc.vector.tensor_scalar(f2[:], pf2[:], wt["b_p2"][:], 0.0,
                                            ALU.add, ALU.max)
                    po = ps_pred.tile([42, BL], F32, tag="pspred")
                    nc.tensor.matmul(po[:], wt["w_po"][:], f2[:],
                                     start=True, stop=True)
                    ot = opool.tile([42, BL], F32, tag="ot")
                    nc.vector.tensor_scalar(ot[:], po[:], wt["b_po"][:], None,
                                            ALU.add)
                    nc.sync.dma_start(out=out_d[t], in_=ot[:])

                st_cur = st_next

    nc.compile()
    return nc


def _fold_weights(inp):
    f = lambda x: np.ascontiguousarray(np.asarray(x), dtype=np.float32)
    w_x = f(inp["proj_w"]) @ f(inp["gru_wih"])          # (34, 192)
    b_x = f(inp["proj_b"]) @ f(inp["gru_wih"]) + f(inp["gru_bih"])  # (192,)
    b_h = f(inp["gru_bhh"])                             # (192,)
    col = lambda v: np.ascontiguousarray(v.reshape(-1, 1), dtype=np.float32)
    return {
        "w_e1": f(inp["enc_w1"]), "b_e1": col(f(inp["enc_b1"])),
        "w_e2": f(inp["enc_w2"]), "b_e2": col(f(inp["enc_b2"])),
        "w_xzr": np.ascontiguousarray(
            np.concatenate([w_x[:, 64:128], w_x[:, :64]], axis=1)),
        "w_xn": f(w_x[:, 128:]),
        "w_hzr": np.ascontiguousarray(np.concatenate(
            [inp["gru_whh"][:, 64:128], inp["gru_whh"][:, :64]],
            axis=1, dtype=np.float32)),
        "w_hn": f(inp["gru_whh"][:, 128:]),
        "b_zr": col(np.concatenate(
            [b_x[64:128] + b_h[64:128], b_x[:64] + b_h[:64]])),
        "b_hn_full": col(np.concatenate([np.zeros(64, np.float32),
                                         b_h[128:]])),
        "b_n": col(b_x[128:]),
        "w_p1h": f(inp["pred_w1"][:64]), "w_p1pe": f(inp["pred_w1"][64:]),
        "b_p1": col(f(inp["pred_b1"])),
        "w_p2": f(inp["pred_w2"]), "b_p2": col(f(inp["pred_b2"])),
        "w_po": np.ascontiguousarray(
            np.concatenate([f(inp["ans_w"]), f(inp["cor_w"])], axis=1)),
        "b_po": col(np.concatenate([f(inp["ans_b"]), f(inp["cor_b"])])),
    }


def _run(inputs, trace=False):
    if "nc" not in _CACHE:
        _CACHE["nc"] = _build_nc()
    nc = _CACHE["nc"]

    wts = _fold_weights(inputs)
    f = lambda x: np.asarray(x, dtype=np.float32)
    probs = f(inputs["problems"])
    ansa = f(inputs["answers"])
    cora = f(inputs["corrects"])

    in_maps = []
    for i in range(NCORES):
        s = slice(i * BL, (i + 1) * BL)
        m = {
            "probsT": np.ascontiguousarray(probs[s].transpose(2, 1, 0)),
            "obsT": np.ascontiguousarray(
                np.stack([ansa[s].T, cora[s].T])),
        }
        m.update(wts)
        in_maps.append(m)

    kw = {}
    if trace:
        import os
        os.makedirs(TRACE_DIR, exist_ok=True)
        kw = {"tmpdir": TRACE_DIR}
    res = run_bass_kernel_spmd(nc, in_maps, core_ids=list(range(NCORES)),
                               trace=trace, **kw)
    outs = [r["out"] for r in res.results]  # each (T, 42, BL)
    ans_logits = np.concatenate(
        [o[:, :41, :].transpose(2, 0, 1) for o in outs], axis=0)
    cor_logits = np.concatenate([o[:, 41, :].T for o in outs], axis=0)
    return (ans_logits, cor_logits), res


def kernel(**inputs):
    (ans_logits, cor_logits), _ = _run(inputs, trace=False)
    return ans_logits, cor_logits


def kernel_traced(**inputs):
    return _run(inputs, trace=True)


# revision 24
# speedup vs baseline: 2.0791x; 2.0791x over previous
"""Trainium2 Bass kernel for nn_ArithmeticUserStateModel.

GRU-based user-state model: B=4096 users x T=256 sequential steps.
Pure data parallel across 8 NeuronCores (512 users per core, weights
replicated). All compute in a transposed layout: feature dims on SBUF
partitions, the 512 local users on the free dim.

Algebraic folding (host-side, exact):
  gates_x = (concat(pe, obs) @ proj_w + proj_b) @ gru_wih + gru_bih
          = peAug @ (proj_w @ gru_wih) + (proj_b @ gru_wih + gru_bih)
so the 34->64 projection matmul never runs on device.
"""

import sys

for _p in ("/opt/trn_rl_repo", "/opt/pypackages"):
    if _p not in sys.path:
        sys.path.insert(0, _p)

import numpy as np

import concourse.bacc as bacc
import concourse.tile as tile
from concourse import mybir
from concourse.bass_utils import run_bass_kernel_spmd

B, T = 4096, 256
NCORES = 8
BL = B // NCORES  # 512 users per core
PD, SD, NB = 32, 64, 41
TC = 8  # time chunk (steps per pipelined chunk)
F32 = mybir.dt.float32
AF = mybir.ActivationFunctionType
ALU = mybir.AluOpType

_CACHE = {}
TRACE_DIR = "/tmp/bass_trace"


def _build_nc():
    nc = bacc.Bacc(debug=False)

    F32R = mybir.dt.float32r
    probs = nc.declare_dram_parameter("probsT", [3, T, BL], F32R, isOutput=False)
    obs = nc.declare_dram_parameter("obsT", [2, T, BL], F32R, isOutput=False)

    wspec = {
        "w_e1": [3, 32], "b_e1": [32, 1],
        "w_e2": [32, 32], "b_e2": [32, 1],
        "w_xrz": [34, 128], "w_xn": [34, 64],
        "w_hrz": [64, 128], "w_hn": [64, 64],
        "b_rz": [128, 1], "b_hn": [64, 1], "b_n": [64, 1],
        "w_p1h": [64, 64], "w_p1pe": [32, 64], "b_p1": [64, 1],
        "w_p2": [64, 64], "b_p2": [64, 1],
        "w_po": [64, 42], "b_po": [42, 1],
    }
    wd = {k: nc.declare_dram_parameter(
              k, s, F32R if k.startswith("w_") else F32, isOutput=False)
          for k, s in wspec.items()}

    out_d = nc.declare_dram_parameter("out", [T, 42, BL], F32, isOutput=True)

    NCH = T // TC

    with tile.TileContext(nc) as tc:
        with (
            tc.tile_pool(name="const", bufs=1) as cpool,
            tc.tile_pool(name="probs", bufs=2) as prob_pool,
            tc.tile_pool(name="peaug", bufs=2) as pe_pool,
            tc.tile_pool(name="states", bufs=3) as st_pool,
            tc.tile_pool(name="work", bufs=2) as work,
            tc.tile_pool(name="outp", bufs=4) as opool,
            tc.tile_pool(name="ps_pe", bufs=2, space="PSUM") as ps_pe,
            tc.tile_pool(name="ps_rz", bufs=2, space="PSUM") as ps_rz,
            tc.tile_pool(name="ps_n", bufs=1, space="PSUM") as ps_n,
            tc.tile_pool(name="ps_pred", bufs=2, space="PSUM") as ps_pred,
        ):
            # ---- load weights/biases to SBUF once ----
            wt = {}
            for k, s in wspec.items():
                t_ = cpool.tile(s, F32R if k.startswith("w_") else F32, tag=k)
                nc.sync.dma_start(out=t_[:], in_=wd[k][:])
                wt[k] = t_

            def mm(out, lhsT, rhs, **kw):
                nc.tensor.matmul(out, lhsT.bitcast(F32R), rhs.bitcast(F32R),
                                 **kw)

            st_cur = st_pool.tile([64, (TC + 1) * BL], F32, tag="states")

            for c in range(NCH):
                t0 = c * TC
                # ---------- Phase A: encoder + peAug chunk ----------
                probT = prob_pool.tile([3, TC * BL], F32R, tag="probT")
                nc.sync.dma_start(
                    out=probT[:],
                    in_=probs[:, t0:t0 + TC, :],
                )
                peaug = pe_pool.tile([34, TC * BL], F32R, tag="peaug")
                nc.sync.dma_start(
                    out=peaug[32:34, :],
                    in_=obs[:, t0:t0 + TC, :],
                )
                for tl in range(TC):
                    sl = slice(tl * BL, (tl + 1) * BL)
                    pe1p = ps_pe.tile([32, BL], F32, tag="psenc")
                    mm(pe1p[:], wt["w_e1"][:], probT[:, sl],
                                     start=True, stop=True)
                    pe1s = work.tile([32, BL], F32R, tag="pe1s")
                    nc.scalar.activation(pe1s[:], pe1p[:], AF.Relu,
                                         bias=wt["b_e1"][:], scale=1.0)
                    pe2p = ps_pe.tile([32, BL], F32, tag="psenc")
                    mm(pe2p[:], wt["w_e2"][:], pe1s[:],
                                     start=True, stop=True)
                    nc.scalar.activation(peaug[0:32, sl], pe2p[:], AF.Relu,
                                         bias=wt["b_e2"][:], scale=1.0)

                st_next = st_pool.tile([64, (TC + 1) * BL], F32, tag="states")

                # ---------- Phase B+C: recurrence + prediction ----------
                for tl in range(TC):
                    t = t0 + tl
                    sl = slice(tl * BL, (tl + 1) * BL)
                    h_prev = st_cur[:, tl * BL:(tl + 1) * BL]

                    # gate pre-activations, gate order [r | z]
                    prz = ps_rz.tile([128, BL], F32, tag="prz")
                    first = (t == 0)
                    mm(prz[:], wt["w_xrz"][:], peaug[:, sl],
                       start=True, stop=first)
                    if not first:
                        mm(prz[:], wt["w_hrz"][:], h_prev,
                           start=False, stop=True)
                    pxn = ps_n.tile([64, BL], F32, tag="pxn")
                    mm(pxn[:], wt["w_xn"][:], peaug[:, sl],
                       start=True, stop=True)
                    if not first:
                        phn = ps_n.tile([64, BL], F32, tag="phn")
                        mm(phn[:], wt["w_hn"][:], h_prev,
                           start=True, stop=True)

                    rz = work.tile([128, BL], F32, tag="rz")
                    nc.scalar.activation(rz[:], prz[:], AF.Sigmoid,
                                         bias=wt["b_rz"][:], scale=1.0)
                    # rhn = (hn + b_hn) * r
                    rhn = work.tile([64, BL], F32, tag="rhn")
                    if first:
                        nc.vector.tensor_scalar_mul(rhn[:], rz[0:64, :],
                                                    wt["b_hn"][:])
                    else:
                        nc.vector.scalar_tensor_tensor(
                            rhn[:], phn[:], wt["b_hn"][:], rz[0:64, :],
                            op0=ALU.add, op1=ALU.mult)
                    sN = work.tile([64, BL], F32, tag="sN")
                    nc.vector.tensor_add(sN[:], rhn[:], pxn[:])
                    nT = work.tile([64, BL], F32, tag="nT")
                    nc.scalar.activation(nT[:], sN[:], AF.Tanh,
                                         bias=wt["b_n"][:], scale=1.0)
                    # h' = n + z*(h - n); d parked at partitions 64:128 so
                    # the z-multiply has base-aligned SBUF inputs, with the
                    # product written back (shifted) to base 0.
                    dhi = work.tile([128, BL], F32, tag="dhi")
                    if first:
                        nc.gpsimd.tensor_scalar_mul(dhi[64:128, :], nT[:],
                                                    -1.0)
                    else:
                        nc.gpsimd.tensor_sub(dhi[64:128, :], h_prev, nT[:])
                    eT = work.tile([64, BL], F32, tag="eT")
                    nc.gpsimd.tensor_mul(eT[:], rz[64:128, :], dhi[64:128, :])
                    if tl < TC - 1:
                        h_dst = st_cur[:, (tl + 1) * BL:(tl + 2) * BL]
                    else:
                        h_dst = st_next[:, 0:BL]
                    nc.gpsimd.tensor_add(h_dst.bitcast(F32R), nT[:], eT[:])

                    # prediction head (uses h_prev)
                    pf1 = ps_pred.tile([64, BL], F32, tag="pspred")
                    mm(pf1[:], wt["w_p1pe"][:], peaug[0:32, sl],
                       start=True, stop=first)
                    if not first:
                        mm(pf1[:], wt["w_p1h"][:], h_prev,
                           start=False, stop=True)
                    f1 = work.tile([64, BL], F32R, tag="f1")
                    nc.vector.tensor_scalar(f1[:], pf1[:], wt["b_p1"][:], 0.0,
                                            ALU.add, ALU.max)
                    pf2 = ps_pred.tile([64, BL], F32, tag="pspred")
                    mm(pf2[:], wt["w_p2"][:], f1[:],
                                     start=True, stop=True)
                    f2 = work.tile([64, BL], F32R, tag="f2")
                    nc.vector.tensor_scalar(f2[:], pf2[:], wt["b_p2"][:], 0.0,
                                            ALU.add, ALU.max)
                    po = ps_pred.tile([42, BL], F32, tag="pspred")
                    mm(po[:], wt["w_po"][:], f2[:],
                                     start=True, stop=True)
                    ot = opool.tile([42, BL], F32, tag="ot")
                    nc.vector.tensor_scalar(ot[:], po[:], wt["b_po"][:], None,
                                            ALU.add)
                    nc.sync.dma_start(out=out_d[t], in_=ot[:])

                st_cur = st_next

    nc.compile()
    return nc


def _fold_weights(inp):
    f = lambda x: np.ascontiguousarray(np.asarray(x), dtype=np.float32)
    w_x = f(inp["proj_w"]) @ f(inp["gru_wih"])          # (34, 192)
    b_x = f(inp["proj_b"]) @ f(inp["gru_wih"]) + f(inp["gru_bih"])  # (192,)
    b_h = f(inp["gru_bhh"])                             # (192,)
    col = lambda v: np.ascontiguousarray(v.reshape(-1, 1), dtype=np.float32)
    return {
        "w_e1": f(inp["enc_w1"]), "b_e1": col(f(inp["enc_b1"])),
        "w_e2": f(inp["enc_w2"]), "b_e2": col(f(inp["enc_b2"])),
        "w_xrz": f(w_x[:, :128]), "w_xn": f(w_x[:, 128:]),
        "w_hrz": f(inp["gru_whh"][:, :128]), "w_hn": f(inp["gru_whh"][:, 128:]),
        "b_rz": col(b_x[:128] + b_h[:128]),
        "b_hn": col(b_h[128:]), "b_n": col(b_x[128:]),
        "w_p1h": f(inp["pred_w1"][:64]), "w_p1pe": f(inp["pred_w1"][64:]),
        "b_p1": col(f(inp["pred_b1"])),
        "w_p2": f(inp["pred_w2"]), "b_p2": col(f(inp["pred_b2"])),
        "w_po": np.ascontiguousarray(
            np.concatenate([f(inp["ans_w"]), f(inp["cor_w"])], axis=1)),
        "b_po": col(np.concatenate([f(inp["ans_b"]), f(inp["cor_b"])])),
    }


def _run(inputs, trace=False):
    if "nc" not in _CACHE:
        _CACHE["nc"] = _build_nc()
    nc = _CACHE["nc"]

    wts = _fold_weights(inputs)
    f = lambda x: np.asarray(x, dtype=np.float32)
    probs = f(inputs["problems"])
    ansa = f(inputs["answers"])
    cora = f(inputs["corrects"])

    in_maps = []
    for i in range(NCORES):
        s = slice(i * BL, (i + 1) * BL)
        m = {
            "probsT": np.ascontiguousarray(probs[s].transpose(2, 1, 0)),
            "obsT": np.ascontiguousarray(
                np.stack([ansa[s].T, cora[s].T])),
        }
        m.update(wts)
        in_maps.append(m)

    kw = {}
    if trace:
        import os, shutil
        shutil.rmtree(TRACE_DIR, ignore_errors=True)
        os.makedirs(TRACE_DIR, exist_ok=True)
        kw = {"tmpdir": TRACE_DIR}
    res = run_bass_kernel_spmd(nc, in_maps, core_ids=list(range(NCORES)),
                               trace=trace, **kw)
    outs = [r["out"] for r in res.results]  # each (T, 42, BL)
    ans_logits = np.concatenate(
        [o[:, :41, :].transpose(2, 0, 1) for o in outs], axis=0)
    cor_logits = np.concatenate([o[:, 41, :].T for o in outs], axis=0)
    return (ans_logits, cor_logits), res


def kernel(**inputs):
    (ans_logits, cor_logits), _ = _run(inputs, trace=False)
    return ans_logits, cor_logits


def kernel_traced(**inputs):
    return _run(inputs, trace=True)


# revision 25
# speedup vs baseline: 2.3412x; 1.1260x over previous
"""Trainium2 Bass kernel for nn_ArithmeticUserStateModel.

GRU-based user-state model: B=4096 users x T=256 sequential steps.
Pure data parallel across 8 NeuronCores (512 users per core, weights
replicated). All compute in a transposed layout: feature dims on SBUF
partitions, the 512 local users on the free dim.

Algebraic folding (host-side, exact):
  gates_x = (concat(pe, obs) @ proj_w + proj_b) @ gru_wih + gru_bih
          = peAug @ (proj_w @ gru_wih) + (proj_b @ gru_wih + gru_bih)
so the 34->64 projection matmul never runs on device.
"""

import sys

for _p in ("/opt/trn_rl_repo", "/opt/pypackages"):
    if _p not in sys.path:
        sys.path.insert(0, _p)

import numpy as np

import concourse.bacc as bacc
import concourse.tile as tile
from concourse import mybir
from concourse.bass_utils import run_bass_kernel_spmd

B, T = 4096, 256
NCORES = 8
BL = B // NCORES  # 512 users per core
PD, SD, NB = 32, 64, 41
TC = 8  # time chunk (steps per pipelined chunk)
HB = 256  # half-batch columns (two interleaved GRU chains)
F32 = mybir.dt.float32
AF = mybir.ActivationFunctionType
ALU = mybir.AluOpType

_CACHE = {}
TRACE_DIR = "/tmp/bass_trace"


def _build_nc():
    nc = bacc.Bacc(debug=False)

    F32R = mybir.dt.float32r
    probs = nc.declare_dram_parameter("probsT", [3, T, BL], F32R, isOutput=False)
    obs = nc.declare_dram_parameter("obsT", [2, T, BL], F32R, isOutput=False)

    wspec = {
        "w_e1": [3, 32], "b_e1": [32, 1],
        "w_e2": [32, 32], "b_e2": [32, 1],
        "w_xrz": [34, 128], "w_xn": [34, 64],
        "w_hrz": [64, 128], "w_hn": [64, 64],
        "b_rz": [128, 1], "b_hn": [64, 1], "b_n": [64, 1],
        "w_p1h": [64, 64], "w_p1pe": [32, 64], "b_p1": [64, 1],
        "w_p2": [64, 64], "b_p2": [64, 1],
        "w_po": [64, 42], "b_po": [42, 1],
    }
    wd = {k: nc.declare_dram_parameter(
              k, s, F32R if k.startswith("w_") else F32, isOutput=False)
          for k, s in wspec.items()}

    out_d = nc.declare_dram_parameter("out", [T, 42, BL], F32, isOutput=True)

    NCH = T // TC

    with tile.TileContext(nc) as tc:
        with (
            tc.tile_pool(name="const", bufs=1) as cpool,
            tc.tile_pool(name="probs", bufs=2) as prob_pool,
            tc.tile_pool(name="peaug", bufs=2) as pe_pool,
            tc.tile_pool(name="states", bufs=3) as st_pool,
            tc.tile_pool(name="work", bufs=2) as work,
            tc.tile_pool(name="outp", bufs=4) as opool,
            tc.tile_pool(name="ps_pe", bufs=2, space="PSUM") as ps_pe,
            tc.tile_pool(name="ps_rz", bufs=1, space="PSUM") as ps_rz,
            tc.tile_pool(name="ps_n", bufs=1, space="PSUM") as ps_n,
            tc.tile_pool(name="ps_pred", bufs=2, space="PSUM") as ps_pred,
        ):
            # ---- load weights/biases to SBUF once ----
            wt = {}
            for k, s in wspec.items():
                t_ = cpool.tile(s, F32R if k.startswith("w_") else F32, tag=k)
                nc.sync.dma_start(out=t_[:], in_=wd[k][:])
                wt[k] = t_

            def mm(out, lhsT, rhs, **kw):
                nc.tensor.matmul(out, lhsT.bitcast(F32R), rhs.bitcast(F32R),
                                 **kw)

            st_cur = st_pool.tile([64, (TC + 1) * BL], F32, tag="states")

            for c in range(NCH):
                t0 = c * TC
                # ---------- Phase A: encoder + peAug chunk ----------
                probT = prob_pool.tile([3, TC * BL], F32R, tag="probT")
                nc.sync.dma_start(
                    out=probT[:],
                    in_=probs[:, t0:t0 + TC, :],
                )
                peaug = pe_pool.tile([34, TC * BL], F32R, tag="peaug")
                nc.sync.dma_start(
                    out=peaug[32:34, :],
                    in_=obs[:, t0:t0 + TC, :],
                )
                for tl in range(TC):
                    sl = slice(tl * BL, (tl + 1) * BL)
                    pe1p = ps_pe.tile([32, BL], F32, tag="psenc")
                    mm(pe1p[:], wt["w_e1"][:], probT[:, sl],
                                     start=True, stop=True)
                    pe1s = work.tile([32, BL], F32R, tag="pe1s")
                    nc.scalar.activation(pe1s[:], pe1p[:], AF.Relu,
                                         bias=wt["b_e1"][:], scale=1.0)
                    pe2p = ps_pe.tile([32, BL], F32, tag="psenc")
                    mm(pe2p[:], wt["w_e2"][:], pe1s[:],
                                     start=True, stop=True)
                    nc.scalar.activation(peaug[0:32, sl], pe2p[:], AF.Relu,
                                         bias=wt["b_e2"][:], scale=1.0)

                st_next = st_pool.tile([64, (TC + 1) * BL], F32, tag="states")

                # ---------- Phase B+C: recurrence + prediction ----------
                # Split into two independent half-batch chains (HB columns
                # each) so the serial GRU dependency chains interleave
                # across engines.
                for tl in range(TC):
                    t = t0 + tl
                    first = (t == 0)
                    for g in range(2):
                        o = tl * BL + g * HB
                        pe_sl = slice(o, o + HB)
                        h_prev = st_cur[:, o:o + HB]
                        gtag = "lo" if g == 0 else "hi"

                        prz = ps_rz.tile([128, HB], F32, tag="prz" + gtag)
                        mm(prz[:], wt["w_xrz"][:], peaug[:, pe_sl],
                           start=True, stop=first)
                        if not first:
                            mm(prz[:], wt["w_hrz"][:], h_prev,
                               start=False, stop=True)
                        # xn and hn share one psum bank: xn at cols 0:HB,
                        # hn at cols HB:2*HB (both dst partition base 0).
                        pn = ps_n.tile([64, 2 * HB], F32, tag="pn" + gtag)
                        mm(pn[:, 0:HB], wt["w_xn"][:], peaug[:, pe_sl],
                           start=True, stop=True)
                        if not first:
                            mm(pn[:, HB:2 * HB], wt["w_hn"][:], h_prev,
                               start=True, stop=True)

                        rz = work.tile([128, HB], F32, tag="rz" + gtag)
                        nc.scalar.activation(rz[:], prz[:], AF.Sigmoid,
                                             bias=wt["b_rz"][:], scale=1.0)
                        # rhn = (hn + b_hn) * r
                        rhn = work.tile([64, HB], F32, tag="rhn" + gtag)
                        if first:
                            nc.vector.tensor_scalar_mul(rhn[:], rz[0:64, :],
                                                        wt["b_hn"][:])
                        else:
                            nc.vector.scalar_tensor_tensor(
                                rhn[:], pn[:, HB:2 * HB], wt["b_hn"][:],
                                rz[0:64, :], op0=ALU.add, op1=ALU.mult)
                        sN = work.tile([64, HB], F32, tag="sN" + gtag)
                        nc.vector.tensor_add(sN[:], rhn[:], pn[:, 0:HB])
                        nT = work.tile([64, HB], F32, tag="nT" + gtag)
                        nc.scalar.activation(nT[:], sN[:], AF.Tanh,
                                             bias=wt["b_n"][:], scale=1.0)
                        # h' = n + z*(h - n); d parked at partitions 64:128
                        # so the z-multiply has base-aligned SBUF inputs.
                        dhi = work.tile([128, HB], F32, tag="dhi" + gtag)
                        if first:
                            nc.gpsimd.tensor_scalar_mul(dhi[64:128, :], nT[:],
                                                        -1.0)
                        else:
                            nc.gpsimd.tensor_sub(dhi[64:128, :], h_prev,
                                                 nT[:])
                        eT = work.tile([64, HB], F32, tag="eT" + gtag)
                        nc.gpsimd.tensor_mul(eT[:], rz[64:128, :],
                                             dhi[64:128, :])
                        if tl < TC - 1:
                            h_dst = st_cur[:, o + BL:o + BL + HB]
                        else:
                            h_dst = st_next[:, g * HB:g * HB + HB]
                        nc.gpsimd.tensor_add(h_dst.bitcast(F32R), nT[:],
                                             eT[:])

                    # prediction head (uses h_prev), also split in halves
                    ot = opool.tile([42, BL], F32, tag="ot")
                    for g in range(2):
                        o = tl * BL + g * HB
                        pe_sl = slice(o, o + HB)
                        h_prev = st_cur[:, o:o + HB]
                        pf1 = ps_pred.tile([64, HB], F32, tag="pspred",
                                           bufs=2)
                        mm(pf1[:], wt["w_p1pe"][:], peaug[0:32, pe_sl],
                           start=True, stop=first)
                        if not first:
                            mm(pf1[:], wt["w_p1h"][:], h_prev,
                               start=False, stop=True)
                        f1 = work.tile([64, HB], F32R, tag="f1")
                        nc.vector.tensor_scalar(f1[:], pf1[:], wt["b_p1"][:],
                                                0.0, ALU.add, ALU.max)
                        pf2 = ps_pred.tile([64, HB], F32, tag="pspred",
                                           bufs=2)
                        mm(pf2[:], wt["w_p2"][:], f1[:], start=True,
                           stop=True)
                        f2 = work.tile([64, HB], F32R, tag="f2")
                        nc.vector.tensor_scalar(f2[:], pf2[:], wt["b_p2"][:],
                                                0.0, ALU.add, ALU.max)
                        po = ps_pred.tile([42, HB], F32, tag="pspred",
                                          bufs=2)
                        mm(po[:], wt["w_po"][:], f2[:], start=True, stop=True)
                        nc.scalar.activation(ot[:, g * HB:g * HB + HB], po[:],
                                             AF.Identity, bias=wt["b_po"][:],
                                             scale=1.0)
                    nc.sync.dma_start(out=out_d[t], in_=ot[:])

                st_cur = st_next

    nc.compile()
    return nc


def _fold_weights(inp):
    f = lambda x: np.ascontiguousarray(np.asarray(x), dtype=np.float32)
    w_x = f(inp["proj_w"]) @ f(inp["gru_wih"])          # (34, 192)
    b_x = f(inp["proj_b"]) @ f(inp["gru_wih"]) + f(inp["gru_bih"])  # (192,)
    b_h = f(inp["gru_bhh"])                             # (192,)
    col = lambda v: np.ascontiguousarray(v.reshape(-1, 1), dtype=np.float32)
    return {
        "w_e1": f(inp["enc_w1"]), "b_e1": col(f(inp["enc_b1"])),
        "w_e2": f(inp["enc_w2"]), "b_e2": col(f(inp["enc_b2"])),
        "w_xrz": f(w_x[:, :128]), "w_xn": f(w_x[:, 128:]),
        "w_hrz": f(inp["gru_whh"][:, :128]), "w_hn": f(inp["gru_whh"][:, 128:]),
        "b_rz": col(b_x[:128] + b_h[:128]),
        "b_hn": col(b_h[128:]), "b_n": col(b_x[128:]),
        "w_p1h": f(inp["pred_w1"][:64]), "w_p1pe": f(inp["pred_w1"][64:]),
        "b_p1": col(f(inp["pred_b1"])),
        "w_p2": f(inp["pred_w2"]), "b_p2": col(f(inp["pred_b2"])),
        "w_po": np.ascontiguousarray(
            np.concatenate([f(inp["ans_w"]), f(inp["cor_w"])], axis=1)),
        "b_po": col(np.concatenate([f(inp["ans_b"]), f(inp["cor_b"])])),
    }


def _run(inputs, trace=False):
    if "nc" not in _CACHE:
        _CACHE["nc"] = _build_nc()
    nc = _CACHE["nc"]

    wts = _fold_weights(inputs)
    f = lambda x: np.asarray(x, dtype=np.float32)
    probs = f(inputs["problems"])
    ansa = f(inputs["answers"])
    cora = f(inputs["corrects"])

    in_maps = []
    for i in range(NCORES):
        s = slice(i * BL, (i + 1) * BL)
        m = {
            "probsT": np.ascontiguousarray(probs[s].transpose(2, 1, 0)),
            "obsT": np.ascontiguousarray(
                np.stack([ansa[s].T, cora[s].T])),
        }
        m.update(wts)
        in_maps.append(m)

    kw = {}
    if trace:
        import os, shutil
        shutil.rmtree(TRACE_DIR, ignore_errors=True)
        os.makedirs(TRACE_DIR, exist_ok=True)
        kw = {"tmpdir": TRACE_DIR}
    res = run_bass_kernel_spmd(nc, in_maps, core_ids=list(range(NCORES)),
                               trace=trace, **kw)
    outs = [r["out"] for r in res.results]  # each (T, 42, BL)
    ans_logits = np.concatenate(
        [o[:, :41, :].transpose(2, 0, 1) for o in outs], axis=0)
    cor_logits = np.concatenate([o[:, 41, :].T for o in outs], axis=0)
    return (ans_logits, cor_logits), res


def kernel(**inputs):
    (ans_logits, cor_logits), _ = _run(inputs, trace=False)
    return ans_logits, cor_logits


def kernel_traced(**inputs):
    return _run(inputs, trace=True)


# revision 29
# speedup vs baseline: 2.3479x; 1.0029x over previous
"""Trainium2 Bass kernel for nn_ArithmeticUserStateModel.

GRU-based user-state model: B=4096 users x T=256 sequential steps.
Pure data parallel across 8 NeuronCores (512 users per core, weights
replicated). All compute in a transposed layout: feature dims on SBUF
partitions, the 512 local users on the free dim.

Algebraic folding (host-side, exact):
  gates_x = (concat(pe, obs) @ proj_w + proj_b) @ gru_wih + gru_bih
          = peAug @ (proj_w @ gru_wih) + (proj_b @ gru_wih + gru_bih)
so the 34->64 projection matmul never runs on device.
"""

import sys

for _p in ("/opt/trn_rl_repo", "/opt/pypackages"):
    if _p not in sys.path:
        sys.path.insert(0, _p)

import numpy as np

import concourse.bacc as bacc
import concourse.tile as tile
from concourse import mybir
from concourse.bass_utils import run_bass_kernel_spmd

B, T = 4096, 256
NCORES = 8
BL = B // NCORES  # 512 users per core
PD, SD, NB = 32, 64, 41
TC = 8  # time chunk (steps per pipelined chunk)
HB = 256  # half-batch columns (two interleaved GRU chains)
F32 = mybir.dt.float32
BF16 = mybir.dt.bfloat16
AF = mybir.ActivationFunctionType
ALU = mybir.AluOpType

_CACHE = {}
TRACE_DIR = "/tmp/bass_trace"


def _build_nc():
    nc = bacc.Bacc(debug=False)

    F32R = mybir.dt.float32r
    probs = nc.declare_dram_parameter("probsT", [3, T, BL], BF16, isOutput=False)
    obs = nc.declare_dram_parameter("obsT", [2, T, BL], BF16, isOutput=False)

    wspec = {
        "w_e1": [3, 32], "b_e1": [32, 1],
        "w_e2": [32, 32], "b_e2": [32, 1],
        "w_xrz": [34, 128], "w_xn": [34, 64],
        "w_hrz": [64, 128], "w_hn": [64, 64],
        "b_rz": [128, 1], "b_hn": [64, 1], "b_n": [64, 1],
        "w_p1h": [64, 64], "w_p1pe": [32, 64], "b_p1": [64, 1],
        "w_p2": [64, 64], "b_p2": [64, 1],
        "w_po": [64, 42], "b_po": [42, 1],
    }
    def _wdt(k):
        if k in ("w_hrz", "w_hn", "w_p1h"):
            return F32R
        if k.startswith("w_"):
            return BF16
        return F32
    wd = {k: nc.declare_dram_parameter(k, s, _wdt(k), isOutput=False)
          for k, s in wspec.items()}

    out_d = nc.declare_dram_parameter("out", [T, 42, BL], F32, isOutput=True)

    NCH = T // TC

    with tile.TileContext(nc) as tc:
        with (
            tc.tile_pool(name="const", bufs=1) as cpool,
            tc.tile_pool(name="probs", bufs=2) as prob_pool,
            tc.tile_pool(name="peaug", bufs=2) as pe_pool,
            tc.tile_pool(name="states", bufs=3) as st_pool,
            tc.tile_pool(name="work", bufs=2) as work,
            tc.tile_pool(name="outp", bufs=4) as opool,
            tc.tile_pool(name="ps_pe", bufs=2, space="PSUM") as ps_pe,
            tc.tile_pool(name="ps_rz", bufs=1, space="PSUM") as ps_rz,
            tc.tile_pool(name="ps_n", bufs=1, space="PSUM") as ps_n,
            tc.tile_pool(name="ps_pred", bufs=2, space="PSUM") as ps_pred,
        ):
            # ---- load weights/biases to SBUF once ----
            wt = {}
            for k, s in wspec.items():
                t_ = cpool.tile(s, _wdt(k), tag=k)
                nc.sync.dma_start(out=t_[:], in_=wd[k][:])
                wt[k] = t_

            def mm(out, lhsT, rhs, **kw):
                nc.tensor.matmul(out, lhsT, rhs, **kw)

            st_cur = st_pool.tile([64, (TC + 1) * BL], F32, tag="states")

            for c in range(NCH):
                t0 = c * TC
                # ---------- Phase A: encoder + peAug chunk ----------
                probT = prob_pool.tile([3, TC * BL], BF16, tag="probT")
                nc.sync.dma_start(
                    out=probT[:],
                    in_=probs[:, t0:t0 + TC, :],
                )
                peaug = pe_pool.tile([34, TC * BL], BF16, tag="peaug")
                nc.sync.dma_start(
                    out=peaug[32:34, :],
                    in_=obs[:, t0:t0 + TC, :],
                )
                for tl in range(TC):
                    sl = slice(tl * BL, (tl + 1) * BL)
                    pe1p = ps_pe.tile([32, BL], F32, tag="psenc")
                    mm(pe1p[:], wt["w_e1"][:], probT[:, sl],
                                     start=True, stop=True)
                    pe1s = work.tile([32, BL], BF16, tag="pe1s")
                    nc.scalar.activation(pe1s[:], pe1p[:], AF.Relu,
                                         bias=wt["b_e1"][:], scale=1.0)
                    pe2p = ps_pe.tile([32, BL], F32, tag="psenc")
                    mm(pe2p[:], wt["w_e2"][:], pe1s[:],
                                     start=True, stop=True)
                    nc.scalar.activation(peaug[0:32, sl], pe2p[:], AF.Relu,
                                         bias=wt["b_e2"][:], scale=1.0)

                st_next = st_pool.tile([64, (TC + 1) * BL], F32, tag="states")

                # ---------- Phase B+C: recurrence + prediction ----------
                # Split into two independent half-batch chains (HB columns
                # each) so the serial GRU dependency chains interleave
                # across engines.
                for tl in range(TC):
                    t = t0 + tl
                    first = (t == 0)
                    for g in range(2):
                        o = tl * BL + g * HB
                        pe_sl = slice(o, o + HB)
                        h_prev = st_cur[:, o:o + HB]
                        gtag = "lo" if g == 0 else "hi"

                        prz = ps_rz.tile([128, HB], F32, tag="prz" + gtag)
                        mm(prz[:], wt["w_xrz"][:], peaug[:, pe_sl],
                           start=True, stop=first)
                        if not first:
                            mm(prz[:], wt["w_hrz"][:],
                               h_prev.bitcast(F32R),
                               start=False, stop=True)
                        # xn and hn share one psum bank: xn at cols 0:HB,
                        # hn at cols HB:2*HB (both dst partition base 0).
                        pn = ps_n.tile([64, 2 * HB], F32, tag="pn" + gtag)
                        mm(pn[:, 0:HB], wt["w_xn"][:], peaug[:, pe_sl],
                           start=True, stop=True)
                        if not first:
                            mm(pn[:, HB:2 * HB], wt["w_hn"][:],
                               h_prev.bitcast(F32R),
                               start=True, stop=True)

                        rz = work.tile([128, HB], F32, tag="rz" + gtag)
                        nc.scalar.activation(rz[:], prz[:], AF.Sigmoid,
                                             bias=wt["b_rz"][:], scale=1.0)
                        # rhn = (hn + b_hn) * r
                        rhn = work.tile([64, HB], F32, tag="rhn" + gtag)
                        if first:
                            nc.vector.tensor_scalar_mul(rhn[:], rz[0:64, :],
                                                        wt["b_hn"][:])
                        else:
                            nc.vector.scalar_tensor_tensor(
                                rhn[:], pn[:, HB:2 * HB], wt["b_hn"][:],
                                rz[0:64, :], op0=ALU.add, op1=ALU.mult)
                        sN = work.tile([64, HB], F32, tag="sN" + gtag)
                        nc.vector.tensor_add(sN[:], rhn[:], pn[:, 0:HB])
                        nT = work.tile([64, HB], F32, tag="nT" + gtag)
                        nc.scalar.activation(nT[:], sN[:], AF.Tanh,
                                             bias=wt["b_n"][:], scale=1.0)
                        # h' = n + z*(h - n); d parked at partitions 64:128
                        # so the z-multiply has base-aligned SBUF inputs.
                        dhi = work.tile([128, HB], F32, tag="dhi" + gtag)
                        if first:
                            nc.gpsimd.tensor_scalar_mul(dhi[64:128, :], nT[:],
                                                        -1.0)
                        else:
                            nc.gpsimd.tensor_sub(dhi[64:128, :], h_prev,
                                                 nT[:])
                        eT = work.tile([64, HB], F32, tag="eT" + gtag)
                        nc.gpsimd.tensor_mul(eT[:], rz[64:128, :],
                                             dhi[64:128, :])
                        if tl < TC - 1:
                            h_dst = st_cur[:, o + BL:o + BL + HB]
                        else:
                            h_dst = st_next[:, g * HB:g * HB + HB]
                        nc.vector.tensor_add(h_dst.bitcast(F32R), nT[:],
                                             eT[:])

                    # prediction head (uses h_prev), also split in halves
                    ot = opool.tile([42, BL], F32, tag="ot")
                    for g in range(2):
                        o = tl * BL + g * HB
                        pe_sl = slice(o, o + HB)
                        h_prev = st_cur[:, o:o + HB]
                        pf1 = ps_pred.tile([64, HB], F32, tag="pspred",
                                           bufs=2)
                        mm(pf1[:], wt["w_p1pe"][:], peaug[0:32, pe_sl],
                           start=True, stop=first)
                        if not first:
                            mm(pf1[:], wt["w_p1h"][:],
                               h_prev.bitcast(F32R),
                               start=False, stop=True)
                        f1 = work.tile([64, HB], BF16, tag="f1")
                        nc.vector.tensor_scalar(f1[:], pf1[:], wt["b_p1"][:],
                                                0.0, ALU.add, ALU.max)
                        pf2 = ps_pred.tile([64, HB], F32, tag="pspred",
                                           bufs=2)
                        mm(pf2[:], wt["w_p2"][:], f1[:], start=True,
                           stop=True)
                        f2 = work.tile([64, HB], BF16, tag="f2")
                        nc.vector.tensor_scalar(f2[:], pf2[:], wt["b_p2"][:],
                                                0.0, ALU.add, ALU.max)
                        po = ps_pred.tile([42, HB], F32, tag="pspred",
                                          bufs=2)
                        mm(po[:], wt["w_po"][:], f2[:], start=True, stop=True)
                        nc.scalar.activation(ot[:, g * HB:g * HB + HB], po[:],
                                             AF.Identity, bias=wt["b_po"][:],
                                             scale=1.0)
                    nc.sync.dma_start(out=out_d[t], in_=ot[:])

                st_cur = st_next

    nc.compile()
    return nc


def _fold_weights(inp):
    import ml_dtypes
    bf16 = ml_dtypes.bfloat16
    f = lambda x: np.ascontiguousarray(np.asarray(x), dtype=np.float32)
    b16 = lambda x: np.ascontiguousarray(np.asarray(x, dtype=np.float32),
                                         dtype=bf16)
    w_x = f(inp["proj_w"]) @ f(inp["gru_wih"])          # (34, 192)
    b_x = f(inp["proj_b"]) @ f(inp["gru_wih"]) + f(inp["gru_bih"])  # (192,)
    b_h = f(inp["gru_bhh"])                             # (192,)
    col = lambda v: np.ascontiguousarray(v.reshape(-1, 1), dtype=np.float32)
    return {
        "w_e1": b16(inp["enc_w1"]), "b_e1": col(f(inp["enc_b1"])),
        "w_e2": b16(inp["enc_w2"]), "b_e2": col(f(inp["enc_b2"])),
        "w_xrz": b16(w_x[:, :128]), "w_xn": b16(w_x[:, 128:]),
        "w_hrz": f(inp["gru_whh"][:, :128]), "w_hn": f(inp["gru_whh"][:, 128:]),
        "b_rz": col(b_x[:128] + b_h[:128]),
        "b_hn": col(b_h[128:]), "b_n": col(b_x[128:]),
        "w_p1h": f(inp["pred_w1"][:64]), "w_p1pe": b16(inp["pred_w1"][64:]),
        "b_p1": col(f(inp["pred_b1"])),
        "w_p2": b16(inp["pred_w2"]), "b_p2": col(f(inp["pred_b2"])),
        "w_po": np.ascontiguousarray(np.concatenate(
            [f(inp["ans_w"]), f(inp["cor_w"])], axis=1), dtype=bf16),
        "b_po": col(np.concatenate([f(inp["ans_b"]), f(inp["cor_b"])])),
    }


def _run(inputs, trace=False):
    if "nc" not in _CACHE:
        _CACHE["nc"] = _build_nc()
    nc = _CACHE["nc"]

    wts = _fold_weights(inputs)
    f = lambda x: np.asarray(x, dtype=np.float32)
    probs = f(inputs["problems"])
    ansa = f(inputs["answers"])
    cora = f(inputs["corrects"])

    import ml_dtypes
    bf16 = ml_dtypes.bfloat16
    in_maps = []
    for i in range(NCORES):
        s = slice(i * BL, (i + 1) * BL)
        m = {
            "probsT": np.ascontiguousarray(
                probs[s].transpose(2, 1, 0), dtype=bf16),
            "obsT": np.ascontiguousarray(
                np.stack([ansa[s].T, cora[s].T]), dtype=bf16),
        }
        m.update(wts)
        in_maps.append(m)

    kw = {}
    if trace:
        import os, shutil
        shutil.rmtree(TRACE_DIR, ignore_errors=True)
        os.makedirs(TRACE_DIR, exist_ok=True)
        kw = {"tmpdir": TRACE_DIR}
    res = run_bass_kernel_spmd(nc, in_maps, core_ids=list(range(NCORES)),
                               trace=trace, **kw)
    outs = [r["out"] for r in res.results]  # each (T, 42, BL)
    ans_logits = np.concatenate(
        [o[:, :41, :].transpose(2, 0, 1) for o in outs], axis=0)
    cor_logits = np.concatenate([o[:, 41, :].T for o in outs], axis=0)
    return (ans_logits, cor_logits), res


def kernel(**inputs):
    (ans_logits, cor_logits), _ = _run(inputs, trace=False)
    return ans_logits, cor_logits


def kernel_traced(**inputs):
    return _run(inputs, trace=True)


# revision 30
# speedup vs baseline: 2.3795x; 1.0134x over previous
"""Trainium2 Bass kernel for nn_ArithmeticUserStateModel.

GRU-based user-state model: B=4096 users x T=256 sequential steps.
Pure data parallel across 8 NeuronCores (512 users per core, weights
replicated). All compute in a transposed layout: feature dims on SBUF
partitions, the 512 local users on the free dim.

Algebraic folding (host-side, exact):
  gates_x = (concat(pe, obs) @ proj_w + proj_b) @ gru_wih + gru_bih
          = peAug @ (proj_w @ gru_wih) + (proj_b @ gru_wih + gru_bih)
so the 34->64 projection matmul never runs on device.
"""

import sys

for _p in ("/opt/trn_rl_repo", "/opt/pypackages"):
    if _p not in sys.path:
        sys.path.insert(0, _p)

import numpy as np

import concourse.bacc as bacc
import concourse.tile as tile
from concourse import mybir
from concourse.bass_utils import run_bass_kernel_spmd

B, T = 4096, 256
NCORES = 8
BL = B // NCORES  # 512 users per core
PD, SD, NB = 32, 64, 41
TC = 8  # time chunk (steps per pipelined chunk)
HB = 256  # half-batch columns (two interleaved GRU chains)
F32 = mybir.dt.float32
FP16 = mybir.dt.float16
AF = mybir.ActivationFunctionType
ALU = mybir.AluOpType

_CACHE = {}
TRACE_DIR = "/tmp/bass_trace"


def _build_nc():
    nc = bacc.Bacc(debug=False)

    F32R = mybir.dt.float32r
    probs = nc.declare_dram_parameter("probsT", [3, T, BL], FP16, isOutput=False)
    obs = nc.declare_dram_parameter("obsT", [2, T, BL], FP16, isOutput=False)

    wspec = {
        "w_e1": [3, 32], "b_e1": [32, 1],
        "w_e2": [32, 32], "b_e2": [32, 1],
        "w_xrz": [34, 128], "w_xn": [34, 64],
        "w_hrz": [64, 128], "w_hn": [64, 64],
        "b_rz": [128, 1], "b_hn": [64, 1], "b_n": [64, 1],
        "w_p1h": [64, 64], "w_p1pe": [32, 64], "b_p1": [64, 1],
        "w_p2": [64, 64], "b_p2": [64, 1],
        "w_po": [64, 42], "b_po": [42, 1],
    }
    def _wdt(k):
        if k.startswith("w_"):
            return FP16
        return F32
    wd = {k: nc.declare_dram_parameter(k, s, _wdt(k), isOutput=False)
          for k, s in wspec.items()}

    out_d = nc.declare_dram_parameter("out", [T, 42, BL], F32, isOutput=True)

    NCH = T // TC

    with tile.TileContext(nc) as tc:
        with (
            tc.tile_pool(name="const", bufs=1) as cpool,
            tc.tile_pool(name="probs", bufs=2) as prob_pool,
            tc.tile_pool(name="peaug", bufs=2) as pe_pool,
            tc.tile_pool(name="states", bufs=3) as st_pool,
            tc.tile_pool(name="work", bufs=2) as work,
            tc.tile_pool(name="outp", bufs=4) as opool,
            tc.tile_pool(name="ps_pe", bufs=2, space="PSUM") as ps_pe,
            tc.tile_pool(name="ps_rz", bufs=1, space="PSUM") as ps_rz,
            tc.tile_pool(name="ps_n", bufs=1, space="PSUM") as ps_n,
            tc.tile_pool(name="ps_pred", bufs=2, space="PSUM") as ps_pred,
        ):
            # ---- load weights/biases to SBUF once ----
            wt = {}
            for k, s in wspec.items():
                t_ = cpool.tile(s, _wdt(k), tag=k)
                nc.sync.dma_start(out=t_[:], in_=wd[k][:])
                wt[k] = t_

            def mm(out, lhsT, rhs, **kw):
                nc.tensor.matmul(out, lhsT, rhs, **kw)

            st_cur = st_pool.tile([64, (TC + 1) * BL], FP16, tag="states")

            for c in range(NCH):
                t0 = c * TC
                # ---------- Phase A: encoder + peAug chunk ----------
                probT = prob_pool.tile([3, TC * BL], FP16, tag="probT")
                nc.sync.dma_start(
                    out=probT[:],
                    in_=probs[:, t0:t0 + TC, :],
                )
                peaug = pe_pool.tile([34, TC * BL], FP16, tag="peaug")
                nc.sync.dma_start(
                    out=peaug[32:34, :],
                    in_=obs[:, t0:t0 + TC, :],
                )
                for tl in range(TC):
                    sl = slice(tl * BL, (tl + 1) * BL)
                    pe1p = ps_pe.tile([32, BL], F32, tag="psenc")
                    mm(pe1p[:], wt["w_e1"][:], probT[:, sl],
                                     start=True, stop=True)
                    pe1s = work.tile([32, BL], FP16, tag="pe1s")
                    nc.scalar.activation(pe1s[:], pe1p[:], AF.Relu,
                                         bias=wt["b_e1"][:], scale=1.0)
                    pe2p = ps_pe.tile([32, BL], F32, tag="psenc")
                    mm(pe2p[:], wt["w_e2"][:], pe1s[:],
                                     start=True, stop=True)
                    nc.scalar.activation(peaug[0:32, sl], pe2p[:], AF.Relu,
                                         bias=wt["b_e2"][:], scale=1.0)

                st_next = st_pool.tile([64, (TC + 1) * BL], FP16, tag="states")

                # ---------- Phase B+C: recurrence + prediction ----------
                # Split into two independent half-batch chains (HB columns
                # each) so the serial GRU dependency chains interleave
                # across engines.
                for tl in range(TC):
                    t = t0 + tl
                    first = (t == 0)
                    for g in range(2):
                        o = tl * BL + g * HB
                        pe_sl = slice(o, o + HB)
                        h_prev = st_cur[:, o:o + HB]
                        gtag = "lo" if g == 0 else "hi"

                        prz = ps_rz.tile([128, HB], F32, tag="prz" + gtag)
                        mm(prz[:], wt["w_xrz"][:], peaug[:, pe_sl],
                           start=True, stop=first)
                        if not first:
                            mm(prz[:], wt["w_hrz"][:], h_prev,
                               start=False, stop=True)
                        # xn and hn share one psum bank: xn at cols 0:HB,
                        # hn at cols HB:2*HB (both dst partition base 0).
                        pn = ps_n.tile([64, 2 * HB], F32, tag="pn" + gtag)
                        mm(pn[:, 0:HB], wt["w_xn"][:], peaug[:, pe_sl],
                           start=True, stop=True)
                        if not first:
                            mm(pn[:, HB:2 * HB], wt["w_hn"][:], h_prev,
                               start=True, stop=True)

                        rz = work.tile([128, HB], FP16, tag="rz" + gtag)
                        nc.scalar.activation(rz[:], prz[:], AF.Sigmoid,
                                             bias=wt["b_rz"][:], scale=1.0)
                        # rhn = (hn + b_hn) * r
                        rhn = work.tile([64, HB], F32, tag="rhn" + gtag)
                        if first:
                            nc.vector.tensor_scalar_mul(rhn[:], rz[0:64, :],
                                                        wt["b_hn"][:])
                        else:
                            nc.vector.scalar_tensor_tensor(
                                rhn[:], pn[:, HB:2 * HB], wt["b_hn"][:],
                                rz[0:64, :], op0=ALU.add, op1=ALU.mult)
                        sN = work.tile([64, HB], F32, tag="sN" + gtag)
                        nc.vector.tensor_add(sN[:], rhn[:], pn[:, 0:HB])
                        nT = work.tile([64, HB], FP16, tag="nT" + gtag)
                        nc.scalar.activation(nT[:], sN[:], AF.Tanh,
                                             bias=wt["b_n"][:], scale=1.0)
                        # h' = n + z*(h - n); d parked at partitions 64:128
                        # so the z-multiply has base-aligned SBUF inputs.
                        dhi = work.tile([128, HB], FP16, tag="dhi" + gtag)
                        if first:
                            nc.gpsimd.tensor_scalar_mul(dhi[64:128, :], nT[:],
                                                        -1.0)
                        else:
                            nc.gpsimd.tensor_sub(dhi[64:128, :], h_prev,
                                                 nT[:])
                        eT = work.tile([64, HB], FP16, tag="eT" + gtag)
                        nc.gpsimd.tensor_mul(eT[:], rz[64:128, :],
                                             dhi[64:128, :])
                        if tl < TC - 1:
                            h_dst = st_cur[:, o + BL:o + BL + HB]
                        else:
                            h_dst = st_next[:, g * HB:g * HB + HB]
                        nc.vector.tensor_add(h_dst, nT[:], eT[:])

                    # prediction head (uses h_prev), also split in halves
                    ot = opool.tile([42, BL], F32, tag="ot")
                    for g in range(2):
                        o = tl * BL + g * HB
                        pe_sl = slice(o, o + HB)
                        h_prev = st_cur[:, o:o + HB]
                        pf1 = ps_pred.tile([64, HB], F32, tag="pspred",
                                           bufs=2)
                        mm(pf1[:], wt["w_p1pe"][:], peaug[0:32, pe_sl],
                           start=True, stop=first)
                        if not first:
                            mm(pf1[:], wt["w_p1h"][:], h_prev,
                               start=False, stop=True)
                        f1 = work.tile([64, HB], FP16, tag="f1")
                        nc.vector.tensor_scalar(f1[:], pf1[:], wt["b_p1"][:],
                                                0.0, ALU.add, ALU.max)
                        pf2 = ps_pred.tile([64, HB], F32, tag="pspred",
                                           bufs=2)
                        mm(pf2[:], wt["w_p2"][:], f1[:], start=True,
                           stop=True)
                        f2 = work.tile([64, HB], FP16, tag="f2")
                        nc.vector.tensor_scalar(f2[:], pf2[:], wt["b_p2"][:],
                                                0.0, ALU.add, ALU.max)
                        po = ps_pred.tile([42, HB], F32, tag="pspred",
                                          bufs=2)
                        mm(po[:], wt["w_po"][:], f2[:], start=True, stop=True)
                        nc.scalar.activation(ot[:, g * HB:g * HB + HB], po[:],
                                             AF.Identity, bias=wt["b_po"][:],
                                             scale=1.0)
                    nc.sync.dma_start(out=out_d[t], in_=ot[:])

                st_cur = st_next

    nc.compile()
    return nc


def _fold_weights(inp):
    f = lambda x: np.ascontiguousarray(np.asarray(x), dtype=np.float32)
    b16 = lambda x: np.ascontiguousarray(np.asarray(x, dtype=np.float32),
                                         dtype=np.float16)
    w_x = f(inp["proj_w"]) @ f(inp["gru_wih"])          # (34, 192)
    b_x = f(inp["proj_b"]) @ f(inp["gru_wih"]) + f(inp["gru_bih"])  # (192,)
    b_h = f(inp["gru_bhh"])                             # (192,)
    col = lambda v: np.ascontiguousarray(v.reshape(-1, 1), dtype=np.float32)
    return {
        "w_e1": b16(inp["enc_w1"]), "b_e1": col(f(inp["enc_b1"])),
        "w_e2": b16(inp["enc_w2"]), "b_e2": col(f(inp["enc_b2"])),
        "w_xrz": b16(w_x[:, :128]), "w_xn": b16(w_x[:, 128:]),
        "w_hrz": b16(inp["gru_whh"][:, :128]),
        "w_hn": b16(inp["gru_whh"][:, 128:]),
        "b_rz": col(b_x[:128] + b_h[:128]),
        "b_hn": col(b_h[128:]), "b_n": col(b_x[128:]),
        "w_p1h": b16(inp["pred_w1"][:64]),
        "w_p1pe": b16(inp["pred_w1"][64:]),
        "b_p1": col(f(inp["pred_b1"])),
        "w_p2": b16(inp["pred_w2"]), "b_p2": col(f(inp["pred_b2"])),
        "w_po": np.ascontiguousarray(np.concatenate(
            [f(inp["ans_w"]), f(inp["cor_w"])], axis=1), dtype=np.float16),
        "b_po": col(np.concatenate([f(inp["ans_b"]), f(inp["cor_b"])])),
    }


def _run(inputs, trace=False):
    if "nc" not in _CACHE:
        _CACHE["nc"] = _build_nc()
    nc = _CACHE["nc"]

    wts = _fold_weights(inputs)
    f = lambda x: np.asarray(x, dtype=np.float32)
    probs = f(inputs["problems"])
    ansa = f(inputs["answers"])
    cora = f(inputs["corrects"])

    in_maps = []
    for i in range(NCORES):
        s = slice(i * BL, (i + 1) * BL)
        m = {
            "probsT": np.ascontiguousarray(
                probs[s].transpose(2, 1, 0), dtype=np.float16),
            "obsT": np.ascontiguousarray(
                np.stack([ansa[s].T, cora[s].T]), dtype=np.float16),
        }
        m.update(wts)
        in_maps.append(m)

    kw = {}
    if trace:
        import os, shutil
        shutil.rmtree(TRACE_DIR, ignore_errors=True)
        os.makedirs(TRACE_DIR, exist_ok=True)
        kw = {"tmpdir": TRACE_DIR}
    res = run_bass_kernel_spmd(nc, in_maps, core_ids=list(range(NCORES)),
                               trace=trace, **kw)
    outs = [r["out"] for r in res.results]  # each (T, 42, BL)
    ans_logits = np.concatenate(
        [o[:, :41, :].transpose(2, 0, 1) for o in outs], axis=0)
    cor_logits = np.concatenate([o[:, 41, :].T for o in outs], axis=0)
    return (ans_logits, cor_logits), res


def kernel(**inputs):
    (ans_logits, cor_logits), _ = _run(inputs, trace=False)
    return ans_logits, cor_logits


def kernel_traced(**inputs):
    return _run(inputs, trace=True)


# revision 31
# speedup vs baseline: 2.3867x; 1.0030x over previous
"""Trainium2 Bass kernel for nn_ArithmeticUserStateModel.

GRU-based user-state model: B=4096 users x T=256 sequential steps.
Pure data parallel across 8 NeuronCores (512 users per core, weights
replicated). All compute in a transposed layout: feature dims on SBUF
partitions, the 512 local users on the free dim.

Algebraic folding (host-side, exact):
  gates_x = (concat(pe, obs) @ proj_w + proj_b) @ gru_wih + gru_bih
          = peAug @ (proj_w @ gru_wih) + (proj_b @ gru_wih + gru_bih)
so the 34->64 projection matmul never runs on device.
"""

import sys

for _p in ("/opt/trn_rl_repo", "/opt/pypackages"):
    if _p not in sys.path:
        sys.path.insert(0, _p)

import numpy as np

import concourse.bacc as bacc
import concourse.tile as tile
from concourse import mybir
from concourse.bass_utils import run_bass_kernel_spmd

B, T = 4096, 256
NCORES = 8
BL = B // NCORES  # 512 users per core
PD, SD, NB = 32, 64, 41
TC = 8  # time chunk (steps per pipelined chunk)
HB = 256  # half-batch columns (two interleaved GRU chains)
F32 = mybir.dt.float32
FP16 = mybir.dt.float16
AF = mybir.ActivationFunctionType
ALU = mybir.AluOpType

_CACHE = {}
TRACE_DIR = "/tmp/bass_trace"


def _build_nc():
    nc = bacc.Bacc(debug=False)

    F32R = mybir.dt.float32r
    probs = nc.declare_dram_parameter("probsT", [3, T, BL], FP16, isOutput=False)
    obs = nc.declare_dram_parameter("obsT", [2, T, BL], FP16, isOutput=False)

    wspec = {
        "w_e1": [3, 32], "b_e1": [32, 1],
        "w_e2": [32, 32], "b_e2": [32, 1],
        "w_xrz": [34, 128], "w_xn": [34, 64],
        "w_hrz": [64, 128], "w_hn": [64, 64],
        "b_rz": [128, 1], "b_hn": [64, 1], "b_n": [64, 1],
        "w_p1h": [64, 64], "w_p1pe": [32, 64], "b_p1": [64, 1],
        "w_p2": [64, 64], "b_p2": [64, 1],
        "w_po": [64, 42], "b_po": [42, 1],
    }
    def _wdt(k):
        if k.startswith("w_"):
            return FP16
        return F32
    wd = {k: nc.declare_dram_parameter(k, s, _wdt(k), isOutput=False)
          for k, s in wspec.items()}

    out_d = nc.declare_dram_parameter("out", [T, 42, BL], F32, isOutput=True)

    NCH = T // TC

    with tile.TileContext(nc) as tc:
        with (
            tc.tile_pool(name="const", bufs=1) as cpool,
            tc.tile_pool(name="probs", bufs=2) as prob_pool,
            tc.tile_pool(name="peaug", bufs=2) as pe_pool,
            tc.tile_pool(name="states", bufs=3) as st_pool,
            tc.tile_pool(name="work", bufs=2) as work,
            tc.tile_pool(name="outp", bufs=4) as opool,
            tc.tile_pool(name="ps_pe", bufs=2, space="PSUM") as ps_pe,
            tc.tile_pool(name="ps_rz", bufs=1, space="PSUM") as ps_rz,
            tc.tile_pool(name="ps_n", bufs=1, space="PSUM") as ps_n,
            tc.tile_pool(name="ps_pred", bufs=2, space="PSUM") as ps_pred,
        ):
            # ---- load weights/biases to SBUF once ----
            wt = {}
            for k, s in wspec.items():
                t_ = cpool.tile(s, _wdt(k), tag=k)
                nc.sync.dma_start(out=t_[:], in_=wd[k][:])
                wt[k] = t_

            def mm(out, lhsT, rhs, **kw):
                nc.tensor.matmul(out, lhsT, rhs, **kw)

            st_cur = st_pool.tile([64, (TC + 1) * BL], FP16, tag="states")

            for c in range(NCH):
                t0 = c * TC
                # ---------- Phase A: encoder + peAug chunk ----------
                probT = prob_pool.tile([3, TC * BL], FP16, tag="probT")
                nc.sync.dma_start(
                    out=probT[:],
                    in_=probs[:, t0:t0 + TC, :],
                )
                peaug = pe_pool.tile([34, TC * BL], FP16, tag="peaug")
                nc.sync.dma_start(
                    out=peaug[32:34, :],
                    in_=obs[:, t0:t0 + TC, :],
                )
                for tl in range(TC):
                    sl = slice(tl * BL, (tl + 1) * BL)
                    pe1p = ps_pe.tile([64, BL], F32, tag="psenc")
                    mm(pe1p[0:32, :], wt["w_e1"][:], probT[:, sl],
                       start=True, stop=True)
                    pe1s = work.tile([32, BL], FP16, tag="pe1s")
                    nc.scalar.activation(pe1s[:], pe1p[0:32, :], AF.Relu,
                                         bias=wt["b_e1"][:], scale=1.0)
                    pe2p = ps_pe.tile([64, BL], F32, tag="psenc")
                    mm(pe2p[32:64, :], wt["w_e2"][:], pe1s[:],
                       start=True, stop=True)
                    nc.scalar.activation(peaug[0:32, sl], pe2p[32:64, :],
                                         AF.Relu,
                                         bias=wt["b_e2"][:], scale=1.0)

                st_next = st_pool.tile([64, (TC + 1) * BL], FP16, tag="states")

                # ---------- Phase B+C: recurrence + prediction ----------
                # Split into two independent half-batch chains (HB columns
                # each) so the serial GRU dependency chains interleave
                # across engines.
                for tl in range(TC):
                    t = t0 + tl
                    first = (t == 0)
                    for g in range(2):
                        o = tl * BL + g * HB
                        pe_sl = slice(o, o + HB)
                        h_prev = st_cur[:, o:o + HB]
                        gtag = "lo" if g == 0 else "hi"

                        prz = ps_rz.tile([128, HB], F32, tag="prz" + gtag)
                        mm(prz[:], wt["w_xrz"][:], peaug[:, pe_sl],
                           start=True, stop=first)
                        if not first:
                            mm(prz[:], wt["w_hrz"][:], h_prev,
                               start=False, stop=True)
                        # xn and hn share one psum bank: xn at cols 0:HB,
                        # hn at cols HB:2*HB (both dst partition base 0).
                        pn = ps_n.tile([64, 2 * HB], F32, tag="pn" + gtag)
                        mm(pn[:, 0:HB], wt["w_xn"][:], peaug[:, pe_sl],
                           start=True, stop=True)
                        if not first:
                            mm(pn[:, HB:2 * HB], wt["w_hn"][:], h_prev,
                               start=True, stop=True)

                        rz = work.tile([128, HB], FP16, tag="rz" + gtag)
                        nc.scalar.activation(rz[:], prz[:], AF.Sigmoid,
                                             bias=wt["b_rz"][:], scale=1.0)
                        # rhn = (hn + b_hn) * r
                        rhn = work.tile([64, HB], F32, tag="rhn" + gtag)
                        if first:
                            nc.vector.tensor_scalar_mul(rhn[:], rz[0:64, :],
                                                        wt["b_hn"][:])
                        else:
                            nc.vector.scalar_tensor_tensor(
                                rhn[:], pn[:, HB:2 * HB], wt["b_hn"][:],
                                rz[0:64, :], op0=ALU.add, op1=ALU.mult)
                        sN = work.tile([64, HB], F32, tag="sN" + gtag)
                        nc.vector.tensor_add(sN[:], rhn[:], pn[:, 0:HB])
                        nT = work.tile([64, HB], FP16, tag="nT" + gtag)
                        nc.scalar.activation(nT[:], sN[:], AF.Tanh,
                                             bias=wt["b_n"][:], scale=1.0)
                        # h' = n + z*(h - n); d parked at partitions 64:128
                        # so the z-multiply has base-aligned SBUF inputs.
                        dhi = work.tile([128, HB], FP16, tag="dhi" + gtag)
                        if first:
                            nc.gpsimd.tensor_scalar_mul(dhi[64:128, :], nT[:],
                                                        -1.0)
                        else:
                            nc.gpsimd.tensor_sub(dhi[64:128, :], h_prev,
                                                 nT[:])
                        eT = work.tile([64, HB], FP16, tag="eT" + gtag)
                        nc.gpsimd.tensor_mul(eT[:], rz[64:128, :],
                                             dhi[64:128, :])
                        if tl < TC - 1:
                            h_dst = st_cur[:, o + BL:o + BL + HB]
                        else:
                            h_dst = st_next[:, g * HB:g * HB + HB]
                        nc.vector.tensor_add(h_dst, nT[:], eT[:])

                    # prediction head (uses h_prev), also split in halves
                    ot = opool.tile([42, BL], F32, tag="ot")
                    for g in range(2):
                        o = tl * BL + g * HB
                        pe_sl = slice(o, o + HB)
                        h_prev = st_cur[:, o:o + HB]
                        cb = 64 * g  # col-group base alternates per half
                        pf1 = ps_pred.tile([128, HB], F32, tag="pspred",
                                           bufs=2)
                        mm(pf1[64:128, :], wt["w_p1pe"][:],
                           peaug[0:32, pe_sl], start=True, stop=first)
                        if not first:
                            mm(pf1[64:128, :], wt["w_p1h"][:], h_prev,
                               start=False, stop=True)
                        f1 = work.tile([64, HB], FP16, tag="f1")
                        nc.vector.tensor_scalar(f1[:], pf1[64:128, :],
                                                wt["b_p1"][:],
                                                0.0, ALU.add, ALU.max)
                        pf2 = ps_pred.tile([128, HB], F32, tag="pspred",
                                           bufs=2)
                        mm(pf2[cb:cb + 64, :], wt["w_p2"][:], f1[:],
                           start=True, stop=True)
                        f2 = work.tile([64, HB], FP16, tag="f2")
                        nc.vector.tensor_scalar(f2[:], pf2[cb:cb + 64, :],
                                                wt["b_p2"][:],
                                                0.0, ALU.add, ALU.max)
                        po = ps_pred.tile([128, HB], F32, tag="pspred",
                                          bufs=2)
                        mm(po[cb:cb + 42, :], wt["w_po"][:], f2[:],
                           start=True, stop=True)
                        nc.scalar.activation(ot[:, g * HB:g * HB + HB],
                                             po[cb:cb + 42, :],
                                             AF.Identity, bias=wt["b_po"][:],
                                             scale=1.0)
                    nc.sync.dma_start(out=out_d[t], in_=ot[:])

                st_cur = st_next

    nc.compile()
    return nc


def _fold_weights(inp):
    f = lambda x: np.ascontiguousarray(np.asarray(x), dtype=np.float32)
    b16 = lambda x: np.ascontiguousarray(np.asarray(x, dtype=np.float32),
                                         dtype=np.float16)
    w_x = f(inp["proj_w"]) @ f(inp["gru_wih"])          # (34, 192)
    b_x = f(inp["proj_b"]) @ f(inp["gru_wih"]) + f(inp["gru_bih"])  # (192,)
    b_h = f(inp["gru_bhh"])                             # (192,)
    col = lambda v: np.ascontiguousarray(v.reshape(-1, 1), dtype=np.float32)
    return {
        "w_e1": b16(inp["enc_w1"]), "b_e1": col(f(inp["enc_b1"])),
        "w_e2": b16(inp["enc_w2"]), "b_e2": col(f(inp["enc_b2"])),
        "w_xrz": b16(w_x[:, :128]), "w_xn": b16(w_x[:, 128:]),
        "w_hrz": b16(inp["gru_whh"][:, :128]),
        "w_hn": b16(inp["gru_whh"][:, 128:]),
        "b_rz": col(b_x[:128] + b_h[:128]),
        "b_hn": col(b_h[128:]), "b_n": col(b_x[128:]),
        "w_p1h": b16(inp["pred_w1"][:64]),
        "w_p1pe": b16(inp["pred_w1"][64:]),
        "b_p1": col(f(inp["pred_b1"])),
        "w_p2": b16(inp["pred_w2"]), "b_p2": col(f(inp["pred_b2"])),
        "w_po": np.ascontiguousarray(np.concatenate(
            [f(inp["ans_w"]), f(inp["cor_w"])], axis=1), dtype=np.float16),
        "b_po": col(np.concatenate([f(inp["ans_b"]), f(inp["cor_b"])])),
    }


def _run(inputs, trace=False):
    if "nc" not in _CACHE:
        _CACHE["nc"] = _build_nc()
    nc = _CACHE["nc"]

    wts = _fold_weights(inputs)
    f = lambda x: np.asarray(x, dtype=np.float32)
    probs = f(inputs["problems"])
    ansa = f(inputs["answers"])
    cora = f(inputs["corrects"])

    in_maps = []
    for i in range(NCORES):
        s = slice(i * BL, (i + 1) * BL)
        m = {
            "probsT": np.ascontiguousarray(
                probs[s].transpose(2, 1, 0), dtype=np.float16),
            "obsT": np.ascontiguousarray(
                np.stack([ansa[s].T, cora[s].T]), dtype=np.float16),
        }
        m.update(wts)
        in_maps.append(m)

    kw = {}
    if trace:
        import os, shutil
        shutil.rmtree(TRACE_DIR, ignore_errors=True)
        os.makedirs(TRACE_DIR, exist_ok=True)
        kw = {"tmpdir": TRACE_DIR}
    res = run_bass_kernel_spmd(nc, in_maps, core_ids=list(range(NCORES)),
                               trace=trace, **kw)
    outs = [r["out"] for r in res.results]  # each (T, 42, BL)
    ans_logits = np.concatenate(
        [o[:, :41, :].transpose(2, 0, 1) for o in outs], axis=0)
    cor_logits = np.concatenate([o[:, 41, :].T for o in outs], axis=0)
    return (ans_logits, cor_logits), res


def kernel(**inputs):
    (ans_logits, cor_logits), _ = _run(inputs, trace=False)
    return ans_logits, cor_logits


def kernel_traced(**inputs):
    return _run(inputs, trace=True)


# revision 33
# speedup vs baseline: 2.5292x; 1.0597x over previous
"""Trainium2 Bass kernel for nn_ArithmeticUserStateModel.

GRU-based user-state model: B=4096 users x T=256 sequential steps.
Pure data parallel across 8 NeuronCores (512 users per core, weights
replicated). All compute in a transposed layout: feature dims on SBUF
partitions, the 512 local users on the free dim, fp16 on the matmul
path (psum accumulation stays fp32).

Key structure:
- One "state" tile per time-chunk holds [h (0:64) | pe (64:96) |
  obs (96:98)] per step, so the gate pre-activation is a single K=98
  matmul and pred-layer-1 a single K=96 matmul (weights concatenated
  host-side; the 34->64 input projection is also folded into the GRU
  input weights algebraically).
- The batch is split into two independent 256-user half-chains that
  interleave across engines to hide the serial GRU dependency.
"""

import sys

for _p in ("/opt/trn_rl_repo", "/opt/pypackages"):
    if _p not in sys.path:
        sys.path.insert(0, _p)

import numpy as np

import concourse.bacc as bacc
import concourse.tile as tile
from concourse import mybir
from concourse.bass_utils import run_bass_kernel_spmd

B, T = 4096, 256
NCORES = 8
BL = B // NCORES  # 512 users per core
PD, SD, NB = 32, 64, 41
TC = 8    # time chunk (steps per pipelined chunk)
HB = 256  # half-batch columns (two interleaved GRU chains)
F32 = mybir.dt.float32
FP16 = mybir.dt.float16
AF = mybir.ActivationFunctionType
ALU = mybir.AluOpType

_CACHE = {}
TRACE_DIR = "/tmp/bass_trace"


def _build_nc():
    nc = bacc.Bacc(debug=False)

    probs = nc.declare_dram_parameter("probsT", [3, T, BL], FP16,
                                      isOutput=False)
    obs = nc.declare_dram_parameter("obsT", [2, T, BL], FP16, isOutput=False)

    wspec = {
        "w_e1": [3, 32], "b_e1": [32, 1],
        "w_e2": [32, 32], "b_e2": [32, 1],
        "w_grz": [98, 128], "w_xn": [34, 64], "w_hn": [64, 64],
        "b_rz": [128, 1], "b_hn": [64, 1], "b_n": [64, 1],
        "w_p1": [96, 64], "b_p1": [64, 1],
        "w_p2": [64, 64], "b_p2": [64, 1],
        "w_po": [64, 42], "b_po": [42, 1],
    }

    def _wdt(k):
        return FP16 if k.startswith("w_") else F32

    wd = {k: nc.declare_dram_parameter(k, s, _wdt(k), isOutput=False)
          for k, s in wspec.items()}

    out_d = nc.declare_dram_parameter("out", [T, 42, BL], F32, isOutput=True)

    NCH = T // TC

    with tile.TileContext(nc) as tc:
        with (
            tc.tile_pool(name="const", bufs=1) as cpool,
            tc.tile_pool(name="probs", bufs=2) as prob_pool,
            tc.tile_pool(name="states", bufs=3) as st_pool,
            tc.tile_pool(name="work", bufs=3) as work,
            tc.tile_pool(name="outp", bufs=4) as opool,
            tc.tile_pool(name="psum", bufs=1, space="PSUM") as psp,
        ):
            wt = {}
            for k, s in wspec.items():
                if k == "w_xn":
                    # its rhs lives at partitions 64:98 of the state tile;
                    # matmul requires lhsT/rhs base partitions to match
                    t_ = cpool.tile([98, s[1]], _wdt(k), tag=k)
                    nc.sync.dma_start(out=t_[64:98, :], in_=wd[k][:])
                    wt[k] = t_[64:98, :]
                else:
                    t_ = cpool.tile(s, _wdt(k), tag=k)
                    nc.sync.dma_start(out=t_[:], in_=wd[k][:])
                    wt[k] = t_

            mm = nc.tensor.matmul

            # state tile per chunk: rows 0:64 h_{t-1}, 64:96 pe[t],
            # 96:98 obs[t]; slot tl <-> free cols [tl*BL, (tl+1)*BL)
            st_cur = st_pool.tile([98, TC * BL], FP16, tag="states")
            nc.vector.memset(st_cur[0:64, 0:BL], 0.0)  # h_{-1} = 0

            for c in range(NCH):
                t0 = c * TC
                # ---------- Phase A: encoder writes pe/obs into state ----
                probT = prob_pool.tile([3, TC * BL], FP16, tag="probT")
                nc.sync.dma_start(out=probT[:], in_=probs[:, t0:t0 + TC, :])
                nc.sync.dma_start(out=st_cur[96:98, :],
                                  in_=obs[:, t0:t0 + TC, :])
                for tl in range(TC):
                    sl = slice(tl * BL, (tl + 1) * BL)
                    pe1p = psp.tile([64, BL], F32, tag="psenc")
                    mm(pe1p[0:32, :], wt["w_e1"][:], probT[:, sl],
                       start=True, stop=True)
                    pe1s = work.tile([32, BL], FP16, tag="pe1s")
                    nc.scalar.activation(pe1s[:], pe1p[0:32, :], AF.Relu,
                                         bias=wt["b_e1"][:], scale=1.0)
                    pe2p = psp.tile([64, BL], F32, tag="psenc")
                    mm(pe2p[32:64, :], wt["w_e2"][:], pe1s[:],
                       start=True, stop=True)
                    nc.scalar.activation(st_cur[64:96, sl], pe2p[32:64, :],
                                         AF.Relu, bias=wt["b_e2"][:],
                                         scale=1.0)

                st_next = st_pool.tile([98, TC * BL], FP16, tag="states")

                # ---------- Phase B+C: recurrence + prediction ----------
                for tl in range(TC):
                    t = t0 + tl
                    # xn for both halves in one full-width matmul
                    pnx = psp.tile([64, BL], F32, tag="pnx")
                    mm(pnx[:], wt["w_xn"],
                       st_cur[64:98, tl * BL:(tl + 1) * BL],
                       start=True, stop=True)

                    for g in range(2):
                        o = tl * BL + g * HB
                        h_prev = st_cur[0:64, o:o + HB]
                        gtag = "lo" if g == 0 else "hi"

                        # r|z pre-activation: single K=98 matmul over
                        # [h | pe | obs]
                        prz = psp.tile([128, HB], F32, tag="prz" + gtag)
                        mm(prz[:], wt["w_grz"][:], st_cur[0:98, o:o + HB],
                           start=True, stop=True)
                        phn = psp.tile([64, HB], F32, tag="phn" + gtag)
                        mm(phn[:], wt["w_hn"][:], h_prev,
                           start=True, stop=True)

                        rz = work.tile([128, HB], FP16, tag="rz" + gtag)
                        nc.scalar.activation(rz[:], prz[:], AF.Sigmoid,
                                             bias=wt["b_rz"][:], scale=1.0)
                        # rhn = (hn + b_hn) * r
                        rhn = work.tile([64, HB], F32, tag="rhn" + gtag)
                        nc.vector.scalar_tensor_tensor(
                            rhn[:], phn[:], wt["b_hn"][:], rz[0:64, :],
                            op0=ALU.add, op1=ALU.mult)
                        sN = work.tile([64, HB], F32, tag="sN" + gtag)
                        nc.vector.tensor_add(sN[:], rhn[:],
                                             pnx[:, g * HB:g * HB + HB])
                        nT = work.tile([64, HB], FP16, tag="nT" + gtag)
                        nc.scalar.activation(nT[:], sN[:], AF.Tanh,
                                             bias=wt["b_n"][:], scale=1.0)
                        # h' = n + z*(h - n); d parked at partitions 64:128
                        # so the z-multiply has base-aligned SBUF inputs.
                        dhi = work.tile([128, HB], FP16, tag="dhi" + gtag)
                        nc.gpsimd.tensor_sub(dhi[64:128, :], h_prev, nT[:])
                        eT = work.tile([64, HB], FP16, tag="eT" + gtag)
                        nc.gpsimd.tensor_mul(eT[:], rz[64:128, :],
                                             dhi[64:128, :])
                        if tl < TC - 1:
                            h_dst = st_cur[0:64, o + BL:o + BL + HB]
                        else:
                            h_dst = st_next[0:64, g * HB:g * HB + HB]
                        nc.vector.tensor_add(h_dst, nT[:], eT[:])

                    # ---- prediction head for step t ----
                    f1 = work.tile([64, BL], FP16, tag="f1")
                    for g in range(2):
                        o = tl * BL + g * HB
                        pf1 = psp.tile([128, HB], F32, tag="pspred", bufs=2)
                        mm(pf1[64:128, :], wt["w_p1"][:],
                           st_cur[0:96, o:o + HB], start=True, stop=True)
                        nc.vector.tensor_scalar(
                            f1[:, g * HB:g * HB + HB], pf1[64:128, :],
                            wt["b_p1"][:], 0.0, ALU.add, ALU.max)
                    pf2 = psp.tile([64, BL], F32, tag="pspred", bufs=2)
                    mm(pf2[:], wt["w_p2"][:], f1[:], start=True, stop=True)
                    f2 = work.tile([64, BL], FP16, tag="f2")
                    nc.vector.tensor_scalar(f2[:], pf2[:], wt["b_p2"][:],
                                            0.0, ALU.add, ALU.max)
                    po = psp.tile([42, BL], F32, tag="pspred", bufs=2)
                    mm(po[:], wt["w_po"][:], f2[:], start=True, stop=True)
                    ot = opool.tile([42, BL], F32, tag="ot")
                    nc.scalar.activation(ot[:], po[:], AF.Identity,
                                         bias=wt["b_po"][:], scale=1.0)
                    nc.sync.dma_start(out=out_d[t], in_=ot[:])

                st_cur = st_next

    nc.compile()
    return nc


def _fold_weights(inp):
    f = lambda x: np.ascontiguousarray(np.asarray(x), dtype=np.float32)
    h16 = lambda x: np.ascontiguousarray(np.asarray(x, dtype=np.float32),
                                         dtype=np.float16)
    w_x = f(inp["proj_w"]) @ f(inp["gru_wih"])          # (34, 192)
    b_x = f(inp["proj_b"]) @ f(inp["gru_wih"]) + f(inp["gru_bih"])  # (192,)
    b_h = f(inp["gru_bhh"])                             # (192,)
    col = lambda v: np.ascontiguousarray(v.reshape(-1, 1), dtype=np.float32)
    w_grz = np.concatenate([f(inp["gru_whh"][:, :128]), w_x[:, :128]])
    return {
        "w_e1": h16(inp["enc_w1"]), "b_e1": col(f(inp["enc_b1"])),
        "w_e2": h16(inp["enc_w2"]), "b_e2": col(f(inp["enc_b2"])),
        "w_grz": h16(w_grz),                      # (98, 128): [h; pe; obs]
        "w_xn": h16(w_x[:, 128:]),
        "w_hn": h16(inp["gru_whh"][:, 128:]),
        "b_rz": col(b_x[:128] + b_h[:128]),
        "b_hn": col(b_h[128:]), "b_n": col(b_x[128:]),
        "w_p1": h16(inp["pred_w1"]),              # (96, 64): [h; pe]
        "b_p1": col(f(inp["pred_b1"])),
        "w_p2": h16(inp["pred_w2"]), "b_p2": col(f(inp["pred_b2"])),
        "w_po": np.ascontiguousarray(np.concatenate(
            [f(inp["ans_w"]), f(inp["cor_w"])], axis=1), dtype=np.float16),
        "b_po": col(np.concatenate([f(inp["ans_b"]), f(inp["cor_b"])])),
    }


def _run(inputs, trace=False):
    if "nc" not in _CACHE:
        _CACHE["nc"] = _build_nc()
    nc = _CACHE["nc"]

    wts = _fold_weights(inputs)
    f = lambda x: np.asarray(x, dtype=np.float32)
    probs = f(inputs["problems"])
    ansa = f(inputs["answers"])
    cora = f(inputs["corrects"])

    in_maps = []
    for i in range(NCORES):
        s = slice(i * BL, (i + 1) * BL)
        m = {
            "probsT": np.ascontiguousarray(
                probs[s].transpose(2, 1, 0), dtype=np.float16),
            "obsT": np.ascontiguousarray(
                np.stack([ansa[s].T, cora[s].T]), dtype=np.float16),
        }
        m.update(wts)
        in_maps.append(m)

    kw = {}
    if trace:
        import os, shutil
        shutil.rmtree(TRACE_DIR, ignore_errors=True)
        os.makedirs(TRACE_DIR, exist_ok=True)
        kw = {"tmpdir": TRACE_DIR}
    res = run_bass_kernel_spmd(nc, in_maps, core_ids=list(range(NCORES)),
                               trace=trace, **kw)
    outs = [r["out"] for r in res.results]  # each (T, 42, BL)
    ans_logits = np.concatenate(
        [o[:, :41, :].transpose(2, 0, 1) for o in outs], axis=0)
    cor_logits = np.concatenate([o[:, 41, :].T for o in outs], axis=0)
    return (ans_logits, cor_logits), res


def kernel(**inputs):
    (ans_logits, cor_logits), _ = _run(inputs, trace=False)
    return ans_logits, cor_logits


def kernel_traced(**inputs):
    return _run(inputs, trace=True)


# revision 34
# speedup vs baseline: 2.7110x; 1.0719x over previous
"""Trainium2 Bass kernel for nn_ArithmeticUserStateModel.

GRU-based user-state model: B=4096 users x T=256 sequential steps.
Pure data parallel across 8 NeuronCores (512 users per core, weights
replicated). All compute in a transposed layout: feature dims on SBUF
partitions, the 512 local users on the free dim, fp16 on the matmul
path (psum accumulation stays fp32).

Key structure:
- One "state" tile per time-chunk holds [h (0:64) | pe (64:96) |
  obs (96:98)] per step, so the gate pre-activation is a single K=98
  matmul and pred-layer-1 a single K=96 matmul (weights concatenated
  host-side; the 34->64 input projection is also folded into the GRU
  input weights algebraically).
- The batch is split into two independent 256-user half-chains that
  interleave across engines to hide the serial GRU dependency.
"""

import sys

for _p in ("/opt/trn_rl_repo", "/opt/pypackages"):
    if _p not in sys.path:
        sys.path.insert(0, _p)

import numpy as np

import concourse.bacc as bacc
import concourse.tile as tile
from concourse import mybir
from concourse.bass_utils import run_bass_kernel_spmd

B, T = 4096, 256
NCORES = 8
BL = B // NCORES  # 512 users per core
PD, SD, NB = 32, 64, 41
TC = 16   # time chunk (steps per pipelined chunk)
HB = 256  # half-batch columns (two interleaved GRU chains)
F32 = mybir.dt.float32
FP16 = mybir.dt.float16
AF = mybir.ActivationFunctionType
ALU = mybir.AluOpType

_CACHE = {}
TRACE_DIR = "/tmp/bass_trace"


def _build_nc():
    nc = bacc.Bacc(debug=False)

    probs = nc.declare_dram_parameter("probsT", [3, T, BL], FP16,
                                      isOutput=False)
    obs = nc.declare_dram_parameter("obsT", [2, T, BL], FP16, isOutput=False)

    wspec = {
        "w_e1": [3, 32], "b_e1": [32, 1],
        "w_e2": [32, 32], "b_e2": [32, 1],
        "w_grz": [98, 128], "w_xn": [34, 64], "w_hn": [64, 64],
        "b_rz": [128, 1], "b_hn": [64, 1], "b_n": [64, 1],
        "w_p1": [96, 64], "b_p1": [64, 1],
        "w_p2": [64, 64], "b_p2": [64, 1],
        "w_po": [64, 42], "b_po": [42, 1],
    }

    def _wdt(k):
        return FP16 if k.startswith("w_") else F32

    wd = {k: nc.declare_dram_parameter(k, s, _wdt(k), isOutput=False)
          for k, s in wspec.items()}

    out_d = nc.declare_dram_parameter("out", [T, 42, BL], F32, isOutput=True)

    NCH = T // TC

    with tile.TileContext(nc) as tc:
        with (
            tc.tile_pool(name="const", bufs=1) as cpool,
            tc.tile_pool(name="probs", bufs=2) as prob_pool,
            tc.tile_pool(name="states", bufs=3) as st_pool,
            tc.tile_pool(name="work", bufs=3) as work,
            tc.tile_pool(name="outp", bufs=4) as opool,
            tc.tile_pool(name="psum", bufs=1, space="PSUM") as psp,
        ):
            wt = {}
            for k, s in wspec.items():
                if k == "w_xn":
                    # its rhs lives at partitions 64:98 of the state tile;
                    # matmul requires lhsT/rhs base partitions to match
                    t_ = cpool.tile([98, s[1]], _wdt(k), tag=k)
                    nc.sync.dma_start(out=t_[64:98, :], in_=wd[k][:])
                    wt[k] = t_[64:98, :]
                else:
                    t_ = cpool.tile(s, _wdt(k), tag=k)
                    nc.sync.dma_start(out=t_[:], in_=wd[k][:])
                    wt[k] = t_

            mm = nc.tensor.matmul

            # state tile per chunk: rows 0:64 h_{t-1}, 64:96 pe[t],
            # 96:98 obs[t]; slot tl <-> free cols [tl*BL, (tl+1)*BL)
            st_cur = st_pool.tile([98, TC * BL], FP16, tag="states")
            nc.vector.memset(st_cur[0:64, 0:BL], 0.0)  # h_{-1} = 0

            for c in range(NCH):
                t0 = c * TC
                # ---------- Phase A: encoder writes pe/obs into state ----
                probT = prob_pool.tile([3, TC * BL], FP16, tag="probT")
                nc.sync.dma_start(out=probT[:], in_=probs[:, t0:t0 + TC, :])
                nc.sync.dma_start(out=st_cur[96:98, :],
                                  in_=obs[:, t0:t0 + TC, :])
                for tl in range(TC):
                    sl = slice(tl * BL, (tl + 1) * BL)
                    pe1p = psp.tile([64, BL], F32, tag="psenc")
                    mm(pe1p[0:32, :], wt["w_e1"][:], probT[:, sl],
                       start=True, stop=True)
                    pe1s = work.tile([32, BL], FP16, tag="pe1s")
                    nc.scalar.activation(pe1s[:], pe1p[0:32, :], AF.Relu,
                                         bias=wt["b_e1"][:], scale=1.0)
                    pe2p = psp.tile([64, BL], F32, tag="psenc")
                    mm(pe2p[32:64, :], wt["w_e2"][:], pe1s[:],
                       start=True, stop=True)
                    nc.scalar.activation(st_cur[64:96, sl], pe2p[32:64, :],
                                         AF.Relu, bias=wt["b_e2"][:],
                                         scale=1.0)

                st_next = st_pool.tile([98, TC * BL], FP16, tag="states")

                # ---------- Phase B+C: recurrence + prediction ----------
                for tl in range(TC):
                    t = t0 + tl
                    # xn for both halves in one full-width matmul
                    pnx = psp.tile([64, BL], F32, tag="pnx")
                    mm(pnx[:], wt["w_xn"],
                       st_cur[64:98, tl * BL:(tl + 1) * BL],
                       start=True, stop=True)

                    for g in range(2):
                        o = tl * BL + g * HB
                        h_prev = st_cur[0:64, o:o + HB]
                        gtag = "lo" if g == 0 else "hi"
                        ctx = tc.high_priority()
                        ctx.__enter__()

                        # r|z pre-activation: single K=98 matmul over
                        # [h | pe | obs]
                        prz = psp.tile([128, HB], F32, tag="prz" + gtag)
                        mm(prz[:], wt["w_grz"][:], st_cur[0:98, o:o + HB],
                           start=True, stop=True)
                        phn = psp.tile([64, HB], F32, tag="phn" + gtag)
                        mm(phn[:], wt["w_hn"][:], h_prev,
                           start=True, stop=True)

                        rz = work.tile([128, HB], FP16, tag="rz" + gtag)
                        nc.scalar.activation(rz[:], prz[:], AF.Sigmoid,
                                             bias=wt["b_rz"][:], scale=1.0)
                        # rhn = (hn + b_hn) * r
                        rhn = work.tile([64, HB], F32, tag="rhn" + gtag)
                        nc.vector.scalar_tensor_tensor(
                            rhn[:], phn[:], wt["b_hn"][:], rz[0:64, :],
                            op0=ALU.add, op1=ALU.mult)
                        sN = work.tile([64, HB], F32, tag="sN" + gtag)
                        nc.vector.tensor_add(sN[:], rhn[:],
                                             pnx[:, g * HB:g * HB + HB])
                        nT = work.tile([64, HB], FP16, tag="nT" + gtag)
                        nc.scalar.activation(nT[:], sN[:], AF.Tanh,
                                             bias=wt["b_n"][:], scale=1.0)
                        # h' = n + z*(h - n); d parked at partitions 64:128
                        # so the z-multiply has base-aligned SBUF inputs.
                        dhi = work.tile([128, HB], FP16, tag="dhi" + gtag)
                        nc.gpsimd.tensor_sub(dhi[64:128, :], h_prev, nT[:])
                        eT = work.tile([64, HB], FP16, tag="eT" + gtag)
                        nc.gpsimd.tensor_mul(eT[:], rz[64:128, :],
                                             dhi[64:128, :])
                        if tl < TC - 1:
                            h_dst = st_cur[0:64, o + BL:o + BL + HB]
                        else:
                            h_dst = st_next[0:64, g * HB:g * HB + HB]
                        nc.vector.tensor_add(h_dst, nT[:], eT[:])
                        ctx.__exit__(None, None, None)

                    # ---- prediction head for step t ----
                    f1 = work.tile([64, BL], FP16, tag="f1")
                    for g in range(2):
                        o = tl * BL + g * HB
                        pf1 = psp.tile([128, HB], F32, tag="pspred", bufs=2)
                        mm(pf1[64:128, :], wt["w_p1"][:],
                           st_cur[0:96, o:o + HB], start=True, stop=True)
                        nc.vector.tensor_scalar(
                            f1[:, g * HB:g * HB + HB], pf1[64:128, :],
                            wt["b_p1"][:], 0.0, ALU.add, ALU.max)
                    pf2 = psp.tile([64, BL], F32, tag="pspred", bufs=2)
                    mm(pf2[:], wt["w_p2"][:], f1[:], start=True, stop=True)
                    f2 = work.tile([64, BL], FP16, tag="f2")
                    nc.vector.tensor_scalar(f2[:], pf2[:], wt["b_p2"][:],
                                            0.0, ALU.add, ALU.max)
                    po = psp.tile([42, BL], F32, tag="pspred", bufs=2)
                    mm(po[:], wt["w_po"][:], f2[:], start=True, stop=True)
                    ot = opool.tile([42, BL], F32, tag="ot")
                    nc.scalar.activation(ot[:], po[:], AF.Identity,
                                         bias=wt["b_po"][:], scale=1.0)
                    nc.sync.dma_start(out=out_d[t], in_=ot[:])

                st_cur = st_next

    nc.compile()
    return nc


def _fold_weights(inp):
    f = lambda x: np.ascontiguousarray(np.asarray(x), dtype=np.float32)
    h16 = lambda x: np.ascontiguousarray(np.asarray(x, dtype=np.float32),
                                         dtype=np.float16)
    w_x = f(inp["proj_w"]) @ f(inp["gru_wih"])          # (34, 192)
    b_x = f(inp["proj_b"]) @ f(inp["gru_wih"]) + f(inp["gru_bih"])  # (192,)
    b_h = f(inp["gru_bhh"])                             # (192,)
    col = lambda v: np.ascontiguousarray(v.reshape(-1, 1), dtype=np.float32)
    w_grz = np.concatenate([f(inp["gru_whh"][:, :128]), w_x[:, :128]])
    return {
        "w_e1": h16(inp["enc_w1"]), "b_e1": col(f(inp["enc_b1"])),
        "w_e2": h16(inp["enc_w2"]), "b_e2": col(f(inp["enc_b2"])),
        "w_grz": h16(w_grz),                      # (98, 128): [h; pe; obs]
        "w_xn": h16(w_x[:, 128:]),
        "w_hn": h16(inp["gru_whh"][:, 128:]),
        "b_rz": col(b_x[:128] + b_h[:128]),
        "b_hn": col(b_h[128:]), "b_n": col(b_x[128:]),
        "w_p1": h16(inp["pred_w1"]),              # (96, 64): [h; pe]
        "b_p1": col(f(inp["pred_b1"])),
        "w_p2": h16(inp["pred_w2"]), "b_p2": col(f(inp["pred_b2"])),
        "w_po": np.ascontiguousarray(np.concatenate(
            [f(inp["ans_w"]), f(inp["cor_w"])], axis=1), dtype=np.float16),
        "b_po": col(np.concatenate([f(inp["ans_b"]), f(inp["cor_b"])])),
    }


def _run(inputs, trace=False):
    if "nc" not in _CACHE:
        _CACHE["nc"] = _build_nc()
    nc = _CACHE["nc"]

    wts = _fold_weights(inputs)
    f = lambda x: np.asarray(x, dtype=np.float32)
    probs = f(inputs["problems"])
    ansa = f(inputs["answers"])
    cora = f(inputs["corrects"])

    in_maps = []
    for i in range(NCORES):
        s = slice(i * BL, (i + 1) * BL)
        m = {
            "probsT": np.ascontiguousarray(
                probs[s].transpose(2, 1, 0), dtype=np.float16),
            "obsT": np.ascontiguousarray(
                np.stack([ansa[s].T, cora[s].T]), dtype=np.float16),
        }
        m.update(wts)
        in_maps.append(m)

    kw = {}
    if trace:
        import os, shutil
        shutil.rmtree(TRACE_DIR, ignore_errors=True)
        os.makedirs(TRACE_DIR, exist_ok=True)
        kw = {"tmpdir": TRACE_DIR}
    res = run_bass_kernel_spmd(nc, in_maps, core_ids=list(range(NCORES)),
                               trace=trace, **kw)
    outs = [r["out"] for r in res.results]  # each (T, 42, BL)
    ans_logits = np.concatenate(
        [o[:, :41, :].transpose(2, 0, 1) for o in outs], axis=0)
    cor_logits = np.concatenate([o[:, 41, :].T for o in outs], axis=0)
    return (ans_logits, cor_logits), res


def kernel(**inputs):
    (ans_logits, cor_logits), _ = _run(inputs, trace=False)
    return ans_logits, cor_logits


def kernel_traced(**inputs):
    return _run(inputs, trace=True)


# revision 35
# speedup vs baseline: 3.0034x; 1.1079x over previous
"""Trainium2 Bass kernel for nn_ArithmeticUserStateModel.

GRU-based user-state model: B=4096 users x T=256 sequential steps.
Pure data parallel across 8 NeuronCores (512 users per core, weights
replicated). All compute in a transposed layout: feature dims on SBUF
partitions, the 512 local users on the free dim, fp16 on the matmul
path (psum accumulation stays fp32).

Key structure:
- One "state" tile per time-chunk holds [h (0:64) | pe (64:96) |
  obs (96:98)] per step, so the gate pre-activation is a single K=98
  matmul and pred-layer-1 a single K=96 matmul (weights concatenated
  host-side; the 34->64 input projection is also folded into the GRU
  input weights algebraically).
- The batch is split into two independent 256-user half-chains that
  interleave across engines to hide the serial GRU dependency.
"""

import sys

for _p in ("/opt/trn_rl_repo", "/opt/pypackages"):
    if _p not in sys.path:
        sys.path.insert(0, _p)

import numpy as np

import concourse.bacc as bacc
import concourse.tile as tile
from concourse import mybir
from concourse.bass_utils import run_bass_kernel_spmd

B, T = 4096, 256
NCORES = 8
BL = B // NCORES  # 512 users per core
PD, SD, NB = 32, 64, 41
TC = 16   # time chunk (steps per pipelined chunk)
HB = 256  # half-batch columns (two interleaved GRU chains)
F32 = mybir.dt.float32
FP16 = mybir.dt.float16
AF = mybir.ActivationFunctionType
ALU = mybir.AluOpType

_CACHE = {}
TRACE_DIR = "/tmp/bass_trace"


def _build_nc():
    nc = bacc.Bacc(debug=False)

    probs = nc.declare_dram_parameter("probsT", [3, T, BL], FP16,
                                      isOutput=False)
    obs = nc.declare_dram_parameter("obsT", [2, T, BL], FP16, isOutput=False)

    wspec = {
        "w_e1": [3, 32], "b_e1": [32, 1],
        "w_e2": [32, 32], "b_e2": [32, 1],
        "w_grz": [98, 128], "w_xn": [34, 64], "w_hn": [64, 64],
        "b_rz": [128, 1], "b_hn": [64, 1], "b_n": [64, 1],
        "w_p1": [96, 64], "b_p1": [64, 1],
        "w_p2": [64, 64], "b_p2": [64, 1],
        "w_po": [64, 42], "b_po": [42, 1],
    }

    def _wdt(k):
        return FP16 if k.startswith("w_") else F32

    wd = {k: nc.declare_dram_parameter(k, s, _wdt(k), isOutput=False)
          for k, s in wspec.items()}

    out_d = nc.declare_dram_parameter("out", [T, 42, BL], F32, isOutput=True)

    NCH = T // TC

    with tile.TileContext(nc) as tc:
        with (
            tc.tile_pool(name="const", bufs=1) as cpool,
            tc.tile_pool(name="probs", bufs=2) as prob_pool,
            tc.tile_pool(name="states", bufs=3) as st_pool,
            tc.tile_pool(name="work", bufs=3) as work,
            tc.tile_pool(name="outp", bufs=4) as opool,
            tc.tile_pool(name="psum", bufs=1, space="PSUM") as psp,
        ):
            wt = {}
            for k, s in wspec.items():
                if k == "w_xn":
                    # its rhs lives at partitions 64:98 of the state tile;
                    # matmul requires lhsT/rhs base partitions to match
                    t_ = cpool.tile([98, s[1]], _wdt(k), tag=k)
                    nc.sync.dma_start(out=t_[64:98, :], in_=wd[k][:])
                    wt[k] = t_[64:98, :]
                else:
                    t_ = cpool.tile(s, _wdt(k), tag=k)
                    nc.sync.dma_start(out=t_[:], in_=wd[k][:])
                    wt[k] = t_

            mm = nc.tensor.matmul

            # state tile per chunk: rows 0:64 h_{t-1}, 64:96 pe[t],
            # 96:98 obs[t]; slot tl <-> free cols [tl*BL, (tl+1)*BL)
            st_cur = st_pool.tile([98, TC * BL], FP16, tag="states")
            nc.vector.memset(st_cur[0:64, 0:BL], 0.0)  # h_{-1} = 0

            for c in range(NCH):
                t0 = c * TC
                # ---------- Phase A: encoder writes pe/obs into state ----
                probT = prob_pool.tile([3, TC * BL], FP16, tag="probT")
                nc.sync.dma_start(out=probT[:], in_=probs[:, t0:t0 + TC, :])
                nc.sync.dma_start(out=st_cur[96:98, :],
                                  in_=obs[:, t0:t0 + TC, :])
                for tl in range(TC):
                    sl = slice(tl * BL, (tl + 1) * BL)
                    pe1p = psp.tile([64, BL], F32, tag="psenc")
                    mm(pe1p[0:32, :], wt["w_e1"][:], probT[:, sl],
                       start=True, stop=True)
                    pe1s = work.tile([32, BL], FP16, tag="pe1s")
                    nc.scalar.activation(pe1s[:], pe1p[0:32, :], AF.Relu,
                                         bias=wt["b_e1"][:], scale=1.0)
                    pe2p = psp.tile([64, BL], F32, tag="psenc")
                    mm(pe2p[32:64, :], wt["w_e2"][:], pe1s[:],
                       start=True, stop=True)
                    nc.scalar.activation(st_cur[64:96, sl], pe2p[32:64, :],
                                         AF.Relu, bias=wt["b_e2"][:],
                                         scale=1.0)

                st_next = st_pool.tile([98, TC * BL], FP16, tag="states")

                # ---------- Phase B+C: recurrence + prediction ----------
                for tl in range(TC):
                    t = t0 + tl
                    # xn for both halves in one full-width matmul
                    pnx = psp.tile([64, BL], F32, tag="pnx")
                    mm(pnx[:], wt["w_xn"],
                       st_cur[64:98, tl * BL:(tl + 1) * BL],
                       start=True, stop=True)

                    for g in range(2):
                        o = tl * BL + g * HB
                        h_prev = st_cur[0:64, o:o + HB]
                        gtag = "lo" if g == 0 else "hi"
                        ctx = tc.high_priority()
                        ctx.__enter__()

                        # r|z pre-activation: single K=98 matmul over
                        # [h | pe | obs]
                        prz = psp.tile([128, HB], F32, tag="prz" + gtag)
                        mm(prz[:], wt["w_grz"][:], st_cur[0:98, o:o + HB],
                           start=True, stop=True)
                        phn = psp.tile([64, HB], F32, tag="phn" + gtag)
                        mm(phn[:], wt["w_hn"][:], h_prev,
                           start=True, stop=True)

                        rz = work.tile([128, HB], FP16, tag="rz" + gtag)
                        nc.scalar.activation(rz[:], prz[:], AF.Sigmoid,
                                             bias=wt["b_rz"][:], scale=1.0)
                        # rhn = (hn + b_hn) * r
                        rhn = work.tile([64, HB], F32, tag="rhn" + gtag)
                        nc.vector.scalar_tensor_tensor(
                            rhn[:], phn[:], wt["b_hn"][:], rz[0:64, :],
                            op0=ALU.add, op1=ALU.mult)
                        sN = work.tile([64, HB], F32, tag="sN" + gtag)
                        nc.vector.tensor_add(sN[:], rhn[:],
                                             pnx[:, g * HB:g * HB + HB])
                        nT = work.tile([64, HB], FP16, tag="nT" + gtag)
                        nc.scalar.activation(nT[:], sN[:], AF.Tanh,
                                             bias=wt["b_n"][:], scale=1.0)
                        # h' = n + z*(h - n); d parked at partitions 64:128
                        # so the z-multiply has base-aligned SBUF inputs.
                        dhi = work.tile([128, HB], FP16, tag="dhi" + gtag)
                        nc.vector.tensor_sub(dhi[64:128, :], h_prev, nT[:])
                        eT = work.tile([64, HB], FP16, tag="eT" + gtag)
                        nc.gpsimd.tensor_mul(eT[:], rz[64:128, :],
                                             dhi[64:128, :])
                        if tl < TC - 1:
                            h_dst = st_cur[0:64, o + BL:o + BL + HB]
                        else:
                            h_dst = st_next[0:64, g * HB:g * HB + HB]
                        nc.vector.tensor_add(h_dst, nT[:], eT[:])
                        ctx.__exit__(None, None, None)

                    # ---- prediction head for step t ----
                    f1 = work.tile([64, BL], FP16, tag="f1")
                    for g in range(2):
                        o = tl * BL + g * HB
                        pf1 = psp.tile([128, HB], F32, tag="pspred", bufs=2)
                        mm(pf1[64:128, :], wt["w_p1"][:],
                           st_cur[0:96, o:o + HB], start=True, stop=True)
                        nc.vector.tensor_scalar(
                            f1[:, g * HB:g * HB + HB], pf1[64:128, :],
                            wt["b_p1"][:], 0.0, ALU.add, ALU.max)
                    pf2 = psp.tile([64, BL], F32, tag="pspred", bufs=2)
                    mm(pf2[:], wt["w_p2"][:], f1[:], start=True, stop=True)
                    f2 = work.tile([64, BL], FP16, tag="f2")
                    nc.vector.tensor_scalar(f2[:], pf2[:], wt["b_p2"][:],
                                            0.0, ALU.add, ALU.max)
                    po = psp.tile([42, BL], F32, tag="pspred", bufs=2)
                    mm(po[:], wt["w_po"][:], f2[:], start=True, stop=True)
                    ot = opool.tile([42, BL], F32, tag="ot")
                    nc.scalar.activation(ot[:], po[:], AF.Identity,
                                         bias=wt["b_po"][:], scale=1.0)
                    nc.sync.dma_start(out=out_d[t], in_=ot[:])

                st_cur = st_next

    nc.compile()
    return nc


def _fold_weights(inp):
    f = lambda x: np.ascontiguousarray(np.asarray(x), dtype=np.float32)
    h16 = lambda x: np.ascontiguousarray(np.asarray(x, dtype=np.float32),
                                         dtype=np.float16)
    w_x = f(inp["proj_w"]) @ f(inp["gru_wih"])          # (34, 192)
    b_x = f(inp["proj_b"]) @ f(inp["gru_wih"]) + f(inp["gru_bih"])  # (192,)
    b_h = f(inp["gru_bhh"])                             # (192,)
    col = lambda v: np.ascontiguousarray(v.reshape(-1, 1), dtype=np.float32)
    w_grz = np.concatenate([f(inp["gru_whh"][:, :128]), w_x[:, :128]])
    return {
        "w_e1": h16(inp["enc_w1"]), "b_e1": col(f(inp["enc_b1"])),
        "w_e2": h16(inp["enc_w2"]), "b_e2": col(f(inp["enc_b2"])),
        "w_grz": h16(w_grz),                      # (98, 128): [h; pe; obs]
        "w_xn": h16(w_x[:, 128:]),
        "w_hn": h16(inp["gru_whh"][:, 128:]),
        "b_rz": col(b_x[:128] + b_h[:128]),
        "b_hn": col(b_h[128:]), "b_n": col(b_x[128:]),
        "w_p1": h16(inp["pred_w1"]),              # (96, 64): [h; pe]
        "b_p1": col(f(inp["pred_b1"])),
        "w_p2": h16(inp["pred_w2"]), "b_p2": col(f(inp["pred_b2"])),
        "w_po": np.ascontiguousarray(np.concatenate(
            [f(inp["ans_w"]), f(inp["cor_w"])], axis=1), dtype=np.float16),
        "b_po": col(np.concatenate([f(inp["ans_b"]), f(inp["cor_b"])])),
    }


def _run(inputs, trace=False):
    if "nc" not in _CACHE:
        _CACHE["nc"] = _build_nc()
    nc = _CACHE["nc"]

    wts = _fold_weights(inputs)
    f = lambda x: np.asarray(x, dtype=np.float32)
    probs = f(inputs["problems"])
    ansa = f(inputs["answers"])
    cora = f(inputs["corrects"])

    in_maps = []
    for i in range(NCORES):
        s = slice(i * BL, (i + 1) * BL)
        m = {
            "probsT": np.ascontiguousarray(
                probs[s].transpose(2, 1, 0), dtype=np.float16),
            "obsT": np.ascontiguousarray(
                np.stack([ansa[s].T, cora[s].T]), dtype=np.float16),
        }
        m.update(wts)
        in_maps.append(m)

    kw = {}
    if trace:
        import os, shutil
        shutil.rmtree(TRACE_DIR, ignore_errors=True)
        os.makedirs(TRACE_DIR, exist_ok=True)
        kw = {"tmpdir": TRACE_DIR}
    res = run_bass_kernel_spmd(nc, in_maps, core_ids=list(range(NCORES)),
                               trace=trace, **kw)
    outs = [r["out"] for r in res.results]  # each (T, 42, BL)
    ans_logits = np.concatenate(
        [o[:, :41, :].transpose(2, 0, 1) for o in outs], axis=0)
    cor_logits = np.concatenate([o[:, 41, :].T for o in outs], axis=0)
    return (ans_logits, cor_logits), res


def kernel(**inputs):
    (ans_logits, cor_logits), _ = _run(inputs, trace=False)
    return ans_logits, cor_logits


def kernel_traced(**inputs):
    return _run(inputs, trace=True)
